# revision 1
# baseline (speedup 1.0000x reference)
"""Trainium2 Bass kernel for a dense transformer attention block.

Reference computation (fp32):
  q = rms_norm(x @ Wq.T)  per head (16 heads x 64)  -> rope -> * q_gain
  k = rms_norm(x @ Wk.T)  per kv-head (4 x 64)      -> rope
  v = x @ Wv.T
  causal GQA attention (16 q heads over 4 kv heads), softmax(q k / 8)
  out = (attn @ v) @ Wo.T

Sharding over 8 cores: core c = 2*b + hh handles batch b (of 4) and
q-head half hh (8 q heads = 2 kv heads).  Each core produces a partial
out [2048, 1024] (its heads' contribution through Wo); the host adds
the two partials per batch.  No collectives.

On-chip layout strategy:
  - host pre-transposes x and the weight slices, so all matmul operands
    arrive with the contraction dim on partitions (no input transposes)
  - scores are built transposed ([k, q]) so softmax needs no P transpose:
    exp(s) with the /8 scale and a -4 shift folded into the ACT op, the
    softmax denominator comes from an extra ones column in v, and the
    normalization is applied per-partition after the PV matmul
  - q_gain is folded into the host-built rope cos/sin tables
  - matmuls run as float32r (full-speed fp32); the attention-probability
    matmul runs in fp16 (safe: exp(s/8-4) <= e^8)
"""

import hashlib
import os

import numpy as np

# The libneuronxla NEFF cache can key-collide across different kernel
# versions with identical I/O shapes (observed: a stale NEFF served for an
# edited kernel).  Key the cache by this file's content so a changed kernel
# never hits a stale entry while identical re-runs stay warm.
try:
    _SRC_HASH = hashlib.sha256(open(__file__, "rb").read()).hexdigest()[:16]
except OSError:
    _SRC_HASH = "nosrc"
os.environ["NEURON_COMPILE_CACHE_URL"] = os.path.join(
    os.environ.get("TMPDIR", "/tmp"), f"neuron-cache-{_SRC_HASH}")

import concourse.bass as bass
import concourse.mybir as mybir
import concourse.tile as tile
from concourse import bacc
from concourse.bass_utils import run_bass_kernel_spmd
from concourse.masks import make_identity, make_upper_triangular

F32 = mybir.dt.float32
F32R = mybir.dt.float32r
F16 = mybir.dt.float16
AFT = mybir.ActivationFunctionType

B, S, D = 4, 2048, 1024
H, HD, KVH = 16, 64, 4
HL = 8            # q heads per core
KVL = 2           # kv heads per core
JQ = HL * HD      # 512 q-proj cols per core
JKV = KVL * HD    # 128 k (or v) proj cols per core
TT = S // 128     # 16 token tiles
DT = D // 128     # 8 contraction tiles
G = 4             # q groups of 512
ROPE_BASE = 10000.0
EPS = 1e-6
N_CORES = 8


def _build_program():
    nc = bacc.Bacc("TRN2", target_bir_lowering=False, debug=False,
                   num_devices=N_CORES)

    xT = nc.dram_tensor("xT", [D, S], F32R, kind="ExternalInput").ap()
    wqT = nc.dram_tensor("wqT", [D, JQ], F32R, kind="ExternalInput").ap()
    wkvT = nc.dram_tensor("wkvT", [D, 2 * JKV], F32R, kind="ExternalInput").ap()
    woT = nc.dram_tensor("woT", [JQ, D], F32R, kind="ExternalInput").ap()
    cosq = nc.dram_tensor("cosq", [S, HL * 32], F32, kind="ExternalInput").ap()
    sinq = nc.dram_tensor("sinq", [S, HL * 32], F32, kind="ExternalInput").ap()
    cosk = nc.dram_tensor("cosk", [S, KVL * 32], F32, kind="ExternalInput").ap()
    sink = nc.dram_tensor("sink", [S, KVL * 32], F32, kind="ExternalInput").ap()
    outp = nc.dram_tensor("outp", [S, D], F32, kind="ExternalOutput").ap()

    with tile.TileContext(nc) as tc:
        with (
            tc.tile_pool(name="consts", bufs=1) as consts,
            tc.tile_pool(name="persist", bufs=1) as persist,
        ):
            ident = consts.tile([128, 128], F32)
            make_identity(nc, ident)
            mask01 = consts.tile([128, 128], F16)
            make_upper_triangular(nc, mask01, val=1.0, diag=True)
            bias_eps = consts.tile([128, 1], F32)
            nc.gpsimd.memset(bias_eps[:], EPS)
            bias_m4 = consts.tile([128, 1], F32)
            nc.gpsimd.memset(bias_m4[:], -4.0)
            bias_0 = consts.tile([128, 1], F32)
            nc.gpsimd.memset(bias_0[:], 0.0)

            wq_sb = persist.tile([128, DT, JQ], F32R)
            wkv_sb = persist.tile([128, DT, 2 * JKV], F32R)
            wo_sb = persist.tile([128, JQ // 128, D], F32R)
            qT_sb = persist.tile([128, JQ // 128, S], F32R)
            kT_sb = persist.tile([128, S], F32R)
            v_sb = persist.tile([128, TT, 2 * (HD + 1)], F16)

            for dt in range(DT):
                nc.sync.dma_start(wkv_sb[:, dt, :], wkvT[128 * dt:128 * (dt + 1), :])
                nc.sync.dma_start(wq_sb[:, dt, :], wqT[128 * dt:128 * (dt + 1), :])

            # ones columns of v (softmax denominator comes out of the PV matmul)
            nc.gpsimd.memset(v_sb[:, :, HD:HD + 1], 1.0)
            nc.gpsimd.memset(v_sb[:, :, 2 * HD + 1:2 * HD + 2], 1.0)

            # ---------------- stage 1: projections, rms, rope, transposes
            with (
                tc.tile_pool(name="xpool", bufs=1) as xpool,
                tc.tile_pool(name="s1", bufs=3) as s1,
                tc.tile_pool(name="ps1", bufs=2, space="PSUM") as ps1,
            ):
                x_sb = xpool.tile([128, DT, S], F32R)
                for dt in range(DT):
                    nc.sync.dma_start(x_sb[:, dt, :], xT[128 * dt:128 * (dt + 1), :])

                for tt in range(TT):
                    tsl = slice(128 * tt, 128 * (tt + 1))

                    psq = ps1.tile([128, JQ], F32, tag="ps_q")
                    pskv = ps1.tile([128, 2 * JKV], F32, tag="ps_kv")
                    for dt in range(DT):
                        lhs = x_sb[:, dt, tsl]
                        nc.tensor.matmul(psq[:], lhs, wq_sb[:, dt, :],
                                         start=(dt == 0), stop=(dt == DT - 1))
                    for dt in range(DT):
                        lhs = x_sb[:, dt, tsl]
                        nc.tensor.matmul(pskv[:], lhs, wkv_sb[:, dt, :],
                                         start=(dt == 0), stop=(dt == DT - 1))

                    # rms statistics for q (8 heads) and k (2 heads).  DVE
                    # can't read PSUM twice in one op, so square against the
                    # SBUF evacuation copy.
                    q_sb = s1.tile([128, JQ], F32, tag="q_sb")
                    nc.scalar.copy(q_sb[:], psq[:])
                    k_sb = s1.tile([128, JKV], F32, tag="k_sb")
                    nc.scalar.copy(k_sb[:], pskv[:, 0:JKV])
                    sq = s1.tile([128, JQ], F32, tag="sq")
                    nc.vector.tensor_mul(sq[:], q_sb[:], psq[:])
                    sk = s1.tile([128, JKV], F32, tag="sk")
                    nc.vector.tensor_mul(sk[:], k_sb[:], pskv[:, 0:JKV])
                    st = s1.tile([128, HL + KVL, 1], F32, tag="st")
                    nc.vector.reduce_sum(out=st[:, 0:HL, :],
                                         in_=sq.rearrange("p (h f) -> p h f", h=HL),
                                         axis=mybir.AxisListType.X)
                    nc.vector.reduce_sum(out=st[:, HL:HL + KVL, :],
                                         in_=sk.rearrange("p (h f) -> p h f", h=KVL),
                                         axis=mybir.AxisListType.X)
                    # r = (mean_sq + eps) ** -0.5 on DVE (Newton; keeps ACT's
                    # table set pinned to exp).  Seed y0 = (1/m)*(a + b*m) has
                    # <= 13% rel err over m in [0.1, 2]; 3 iterations -> ~1e-6.
                    mm = s1.tile([128, HL + KVL, 1], F32, tag="mm")
                    nc.vector.tensor_scalar(mm[:], st[:], 1.0 / HD, EPS,
                                            mybir.AluOpType.mult,
                                            mybir.AluOpType.add)
                    st_w = s1.tile([128, HL + KVL, 1], F32, tag="st_w")
                    nc.vector.reciprocal(st_w[:], mm[:])
                    st_r = s1.tile([128, HL + KVL, 1], F32, tag="st_r")
                    nc.vector.tensor_scalar(st_r[:], mm[:], 0.657, 0.294,
                                            mybir.AluOpType.mult,
                                            mybir.AluOpType.add)
                    nc.vector.tensor_mul(st_r[:], st_r[:], st_w[:])
                    nt = s1.tile([128, HL + KVL, 1], F32, tag="nt")
                    for _ in range(3):
                        nc.vector.tensor_mul(nt[:], st_r[:], st_r[:])
                        nc.vector.tensor_mul(nt[:], nt[:], mm[:])
                        nc.vector.tensor_scalar(nt[:], nt[:], -0.5, 1.5,
                                                mybir.AluOpType.mult,
                                                mybir.AluOpType.add)
                        nc.vector.tensor_mul(st_r[:], st_r[:], nt[:])

                    # scale q/k by their rms factors (in place)
                    for h in range(HL):
                        nc.vector.tensor_scalar_mul(
                            q_sb[:, 64 * h:64 * (h + 1)],
                            q_sb[:, 64 * h:64 * (h + 1)], st_r[:, h, :])
                    for u in range(KVL):
                        nc.vector.tensor_scalar_mul(
                            k_sb[:, 64 * u:64 * (u + 1)],
                            k_sb[:, 64 * u:64 * (u + 1)], st_r[:, HL + u, :])
                    # v -> fp16 slots (per kv head: 64 values + the ones col)
                    for u in range(KVL):
                        nc.scalar.copy(
                            v_sb[:, tt, (HD + 1) * u:(HD + 1) * u + HD],
                            pskv[:, JKV + 64 * u:JKV + 64 * (u + 1)])

                    # rope tables for this token tile
                    cq = s1.tile([128, HL * 32], F32, tag="cq")
                    nc.sync.dma_start(cq[:], cosq[tsl, :])
                    sq_t = s1.tile([128, HL * 32], F32, tag="sq_t")
                    nc.sync.dma_start(sq_t[:], sinq[tsl, :])
                    ck = s1.tile([128, KVL * 32], F32, tag="ck")
                    nc.sync.dma_start(ck[:], cosk[tsl, :])
                    sk_t = s1.tile([128, KVL * 32], F32, tag="sk_t")
                    nc.sync.dma_start(sk_t[:], sink[tsl, :])

                    def rope(dst, src, cos_t, sin_t, nh, tmp):
                        s3 = src.rearrange("p (h f) -> p h f", h=nh)
                        d3 = dst.rearrange("p (h f) -> p h f", h=nh)
                        c3 = cos_t.rearrange("p (h f) -> p h f", h=nh)
                        n3 = sin_t.rearrange("p (h f) -> p h f", h=nh)
                        t1, t2 = s3[:, :, 0:32], s3[:, :, 32:64]
                        o1, o2 = d3[:, :, 0:32], d3[:, :, 32:64]
                        nc.vector.tensor_mul(o1, t1, c3)
                        nc.vector.tensor_mul(tmp[:], t2, n3)
                        nc.vector.tensor_add(o1, o1, tmp[:])
                        nc.vector.tensor_mul(o2, t2, c3)
                        nc.vector.tensor_mul(tmp[:], t1, n3)
                        nc.vector.tensor_sub(o2, o2, tmp[:])

                    qr = s1.tile([128, JQ], F32, tag="qr")
                    tmpq = s1.tile([128, HL, 32], F32, tag="tmpq")
                    rope(qr, q_sb, cq, sq_t, HL, tmpq)
                    kr = s1.tile([128, JKV], F32, tag="kr")
                    tmpk = s1.tile([128, KVL, 32], F32, tag="tmpk")
                    rope(kr, k_sb, ck, sk_t, KVL, tmpk)

                    # transpose q, k into feature-major layout.  q head h goes
                    # to qT_sb[64*(h//4) : +64, h%4, :] so its partition offset
                    # matches its kv head's offset in kT_sb (matmul requires
                    # equal base partitions for lhsT and rhs).
                    # q head SLOTS are host-permuted to [0,4,1,5,2,6,3,7] so a
                    # contiguous 128-col block holds the two heads that share
                    # a kv head at partition offsets {0, 64}.
                    for hp in range(4):
                        ptr = ps1.tile([128, 128], F32, tag="ps_tr")
                        nc.tensor.transpose(ptr[:], qr[:, 128 * hp:128 * (hp + 1)],
                                            ident[:])
                        nc.vector.tensor_copy(qT_sb[:, hp, tsl], ptr[:])
                    ptk = ps1.tile([128, 128], F32, tag="ps_tk", bufs=1)
                    nc.tensor.transpose(ptk[:], kr[:], ident[:])
                    nc.vector.tensor_copy(kT_sb[:, tsl], ptk[:])

            # ---------------- stage 2+3: attention + output projection
            for ft in range(JQ // 128):
                nc.sync.dma_start(wo_sb[:, ft, :], woT[128 * ft:128 * (ft + 1), :])
            with (
                tc.tile_pool(name="s2", bufs=2) as s2,
                tc.tile_pool(name="ps_s", bufs=2, space="PSUM") as psum_s,
                tc.tile_pool(name="ps_y", bufs=2, space="PSUM") as psum_y,
                tc.tile_pool(name="ps_t", bufs=1, space="PSUM") as psum_t,
                tc.tile_pool(name="ps_o", bufs=1, space="PSUM") as psum_o,
            ):
                for g in range(G):
                    y_sb = s2.tile([128, 4, JQ], F32, tag="y_sb")
                    for h in range(HL):
                        # h is the SLOT index; its kv head is u = h % 2 and it
                        # lives at qT_sb[64*(h%2) : +64, h//2, :]
                        u = h % 2
                        qrhs = qT_sb[64 * u:64 * (u + 1), h // 2,
                                     512 * g:512 * (g + 1)]
                        expT = s2.tile([128, 4 * g + 4, 512], F16, tag="expT")

                        # full rectangle k-tiles, 2 per PSUM batch (2 banks x
                        # bufs=2 keeps QK matmuls and exp evacuation pipelined)
                        for c in range(2 * g):
                            pss = psum_s.tile([128, 2, 512], F32, tag="pss")
                            for lane in range(2):
                                kt = 2 * c + lane
                                nc.tensor.matmul(
                                    pss[:, lane, :],
                                    kT_sb[64 * u:64 * (u + 1),
                                          128 * kt:128 * (kt + 1)],
                                    qrhs)
                            nc.scalar.activation(expT[:, 2 * c:2 * c + 2, :],
                                                 pss[:], AFT.Exp,
                                                 scale=0.125, bias=bias_m4[:])
                        # diagonal k-tiles (causal frontier)
                        for dc in range(2):
                            pss = psum_s.tile([128, 2, 512], F32, tag="pss")
                            for lane in range(2):
                                kt = 4 * g + 2 * dc + lane
                                n0 = 128 * (2 * dc + lane)
                                nc.tensor.matmul(
                                    pss[:, lane, n0:512],
                                    kT_sb[64 * u:64 * (u + 1),
                                          128 * kt:128 * (kt + 1)],
                                    qrhs[:, n0:512])
                                nc.scalar.activation(expT[:, kt, n0:512],
                                                     pss[:, lane, n0:512], AFT.Exp,
                                                     scale=0.125, bias=bias_m4[:])
                                nc.vector.tensor_mul(expT[:, kt, n0:n0 + 128],
                                                     expT[:, kt, n0:n0 + 128],
                                                     mask01[:])

                        # PV: y[q, f] (+ softmax denominator in the last col)
                        for i in range(4):
                            nkt = 4 * g + i + 1
                            psy = psum_y.tile([128, HD + 1], F32, tag="psy")
                            for kt in range(nkt):
                                nc.tensor.matmul(
                                    psy[:],
                                    expT[:, kt, 128 * i:128 * (i + 1)],
                                    v_sb[:, kt, (HD + 1) * u:(HD + 1) * (u + 1)],
                                    start=(kt == 0), stop=(kt == nkt - 1))
                            lc = s2.tile([128, 1], F32, tag="lc")
                            nc.vector.tensor_copy(lc[:], psy[:, HD:HD + 1])
                            rl = s2.tile([128, 1], F32, tag="rl")
                            nc.vector.reciprocal(rl[:], lc[:])
                            nc.vector.tensor_scalar_mul(
                                y_sb[:, i, 64 * h:64 * (h + 1)],
                                psy[:, 0:HD], rl[:])

                    # output projection for this group of 512 rows
                    for i in range(4):
                        yT = s2.tile([128, JQ // 128, 128], F32R, tag="yT")
                        for ft in range(JQ // 128):
                            ptt = psum_t.tile([128, 128], F32, tag="ptt")
                            nc.tensor.transpose(
                                ptt[:], y_sb[:, i, 128 * ft:128 * (ft + 1)],
                                ident[:])
                            nc.vector.tensor_copy(yT[:, ft, :], ptt[:])
                        out_sb = s2.tile([128, D], F32, tag="out_sb")
                        for nt in range(D // 512):
                            pso = psum_o.tile([128, 512], F32, tag="pso")
                            for ft in range(JQ // 128):
                                nc.tensor.matmul(
                                    pso[:], yT[:, ft, :],
                                    wo_sb[:, ft, 512 * nt:512 * (nt + 1)],
                                    start=(ft == 0), stop=(ft == JQ // 128 - 1))
                            nc.vector.tensor_copy(out_sb[:, 512 * nt:512 * (nt + 1)],
                                                  pso[:])
                        r0 = 512 * g + 128 * i
                        nc.sync.dma_start(outp[r0:r0 + 128, :], out_sb[:])

    nc.compile()
    return nc


_PROGRAM_CACHE = {}


def _rope_tables(gains):
    inv_freq = 1.0 / (ROPE_BASE ** (np.arange(0, HD, 2, dtype=np.float32) / HD))
    t = np.arange(S, dtype=np.float32)
    freqs = np.outer(t, inv_freq)                    # [S, 32]
    cos, sin = np.cos(freqs), np.sin(freqs)
    cos_g = np.concatenate([cos * g for g in gains], axis=1).astype(np.float32)
    sin_g = np.concatenate([sin * g for g in gains], axis=1).astype(np.float32)
    return np.ascontiguousarray(cos_g), np.ascontiguousarray(sin_g)


# q-head slot order: slot s holds local head PERM[s], so a contiguous
# 128-col block pairs the two heads sharing a kv head (see _build_program)
PERM = [0, 4, 1, 5, 2, 6, 3, 7]


def _in_map_for_core(x, Wq, Wk, Wv, Wo, q_gain, core):
    rows = np.concatenate([np.arange(64 * p, 64 * (p + 1)) for p in PERM])
    b, hh = core // 2, core % 2
    qh0 = JQ * hh
    kvh = slice(JKV * hh, JKV * (hh + 1))     # kv rows of Wk/Wv
    wkv = np.concatenate([Wk[kvh, :], Wv[kvh, :]], axis=0)  # [256, 1024]
    gains = q_gain[HL * hh:HL * (hh + 1)][PERM]
    cq, sq = _rope_tables(gains)
    ck, sk = _rope_tables(np.ones(KVL, dtype=np.float32))
    return {
        "xT": np.ascontiguousarray(x[b].T),
        "wqT": np.ascontiguousarray(Wq[qh0 + rows, :].T),
        "wkvT": np.ascontiguousarray(wkv.T),
        "woT": np.ascontiguousarray(Wo[:, qh0 + rows].T),
        "cosq": cq, "sinq": sq, "cosk": ck, "sink": sk,
    }


def kernel(x, Wq, Wk, Wv, Wo, q_gain):
    x = np.asarray(x, dtype=np.float32)
    Wq = np.asarray(Wq, dtype=np.float32)
    Wk = np.asarray(Wk, dtype=np.float32)
    Wv = np.asarray(Wv, dtype=np.float32)
    Wo = np.asarray(Wo, dtype=np.float32)
    q_gain = np.asarray(q_gain, dtype=np.float32)

    if "nc" not in _PROGRAM_CACHE:
        _PROGRAM_CACHE["nc"] = _build_program()
    nc = _PROGRAM_CACHE["nc"]

    in_maps = [_in_map_for_core(x, Wq, Wk, Wv, Wo, q_gain, core)
               for core in range(N_CORES)]

    res = run_bass_kernel_spmd(nc, in_maps, core_ids=list(range(N_CORES)))
    _PROGRAM_CACHE["last_results"] = res

    out = np.empty((B, S, D), dtype=np.float32)
    for b in range(B):
        out[b] = res.results[2 * b]["outp"] + res.results[2 * b + 1]["outp"]
    return out


if __name__ == "__main__":
    rng = np.random.default_rng(0)
    inputs = {
        "x": rng.standard_normal((B, S, D), dtype=np.float32),
        "Wq": rng.standard_normal((D, D), dtype=np.float32) * 0.02,
        "Wk": rng.standard_normal((KVH * HD, D), dtype=np.float32) * 0.02,
        "Wv": rng.standard_normal((KVH * HD, D), dtype=np.float32) * 0.02,
        "Wo": rng.standard_normal((D, D), dtype=np.float32) * 0.02,
        "q_gain": np.full((H,), 1.5, dtype=np.float32),
    }
    out = kernel(**inputs)
    print(out.shape, out.dtype, np.abs(out).max())



# revision 37
# speedup vs baseline: 1.1915x; 1.1915x over previous
"""Trainium2 Bass kernel for a dense transformer attention block.

Reference computation (fp32):
  q = rms_norm(x @ Wq.T)  per head (16 heads x 64)  -> rope -> * q_gain
  k = rms_norm(x @ Wk.T)  per kv-head (4 x 64)      -> rope
  v = x @ Wv.T
  causal GQA attention (16 q heads over 4 kv heads), softmax(q k / 8)
  out = (attn @ v) @ Wo.T

Sharding over 8 cores: core c = 2*b + hh handles batch b (of 4) and
q-head half hh (8 q heads = 2 kv heads).  Each core produces a partial
out [2048, 1024] (its heads' contribution through Wo); the host adds
the two partials per batch.  No collectives.

Datapath design (v2, tuned against the TimelineSim cost model):
  - projections run in fp32r (full speed at free-size>=256); everything
    downstream (q/k/v, rope tables, probabilities, y, Wo) is fp16, which
    doubles DVE throughput (2x_1p) and keeps all matmuls at 1 cycle/row
  - q/k features are pair-interleaved (slot 2m <- feat m, 2m+1 <- feat
    32+m) so rope is 3 wide DVE ops using a stride -1 pair-swap view;
    the rotation sign lives in the host-built sin table; q_gain is
    folded into the q cos/sin tables
  - RMS statistics come from one bn_stats op per projection (even/odd
    mean/M2 6-tuples), assembled and pushed through a batched Newton
    rsqrt over 4 token tiles at a time (all fp32, ~13 small DVE ops)
  - scores are built transposed ([k, q]) so softmax needs no transposes:
    exp(s/8 - 4) is fused into the ACT evacuation of the QK psum (2
    k-tiles per instruction), the denominator comes from a ones column
    in v, and the normalization is applied per-partition post-PV
  - causal frontier masking runs on the otherwise-idle GPSIMD engine
  - stage-2 issues QK(s) one head ahead of PV(s-1) so the scalar engine
    (the global bottleneck: ~157us of exp) never starves
"""

import hashlib
import os

import numpy as np

# The libneuronxla NEFF cache can key-collide across different kernel
# versions with identical I/O shapes (observed: a stale NEFF served for an
# edited kernel).  Key the cache by this file's content so a changed kernel
# never hits a stale entry while identical re-runs stay warm.
try:
    _SRC_HASH = hashlib.sha256(open(__file__, "rb").read()).hexdigest()[:16]
except OSError:
    _SRC_HASH = "nosrc"
os.environ["NEURON_COMPILE_CACHE_URL"] = os.path.join(
    os.environ.get("TMPDIR", "/tmp"), f"neuron-cache-{_SRC_HASH}")

import concourse.bass as bass
import concourse.mybir as mybir
import concourse.tile as tile
from concourse import bacc
from concourse.bass_utils import run_bass_kernel_spmd
from concourse.masks import make_identity, make_upper_triangular

F32 = mybir.dt.float32
F32R = mybir.dt.float32r
F16 = mybir.dt.float16
AFT = mybir.ActivationFunctionType
ALU = mybir.AluOpType

B, S, D = 4, 2048, 1024
H, HD, KVH = 16, 64, 4
HL = 8            # q heads per core
KVL = 2           # kv heads per core
JQ = HL * HD      # 512 q-proj cols per core
JKV = KVL * HD    # 128 k (or v) proj cols per core
TT = S // 128     # 16 token tiles
DT = D // 128     # 8 contraction tiles
G = 4             # q groups of 512
ROPE_BASE = 10000.0
EPS = 1e-6
N_CORES = 8


def _build_program():
    nc = bacc.Bacc("TRN2", target_bir_lowering=False, debug=False,
                   num_devices=N_CORES)

    xT = nc.dram_tensor("xT", [D, S], F32R, kind="ExternalInput").ap()
    wqT = nc.dram_tensor("wqT", [D, JQ], F32R, kind="ExternalInput").ap()
    wkvT = nc.dram_tensor("wkvT", [D, 2 * JKV], F32R, kind="ExternalInput").ap()
    woT = nc.dram_tensor("woT", [JQ, D], F16, kind="ExternalInput").ap()
    cqi = nc.dram_tensor("cqi", [S, JQ], F16, kind="ExternalInput").ap()
    sqi = nc.dram_tensor("sqi", [S, JQ], F16, kind="ExternalInput").ap()
    cki = nc.dram_tensor("cki", [S, JKV], F16, kind="ExternalInput").ap()
    ski = nc.dram_tensor("ski", [S, JKV], F16, kind="ExternalInput").ap()
    outp = nc.dram_tensor("outp", [S, D], F32, kind="ExternalOutput").ap()
    xTr = xT.rearrange("(dt p) s -> p dt s", p=128)

    with tile.TileContext(nc) as tc:
        with (
            tc.tile_pool(name="consts", bufs=1) as consts,
            tc.tile_pool(name="persist", bufs=1) as persist,
        ):
            ident = consts.tile([128, 128], F16)
            make_identity(nc, ident)
            mask01 = consts.tile([128, 128], F16)
            make_upper_triangular(nc, mask01, val=1.0, diag=True)
            bias_m4 = consts.tile([128, 1], F32)
            nc.gpsimd.memset(bias_m4[:], -4.0)

            wq_sb = persist.tile([128, DT, JQ], F32R)
            wkv_sb = persist.tile([128, DT, 2 * JKV], F32R)
            wo_sb = persist.tile([128, JQ // 128, D], F16)
            qT_sb = persist.tile([128, 4, S], F16)
            kT_sb = persist.tile([128, S], F16)
            v_sb = persist.tile([128, TT, 2 * (HD + 1)], F16)
            stats = persist.tile([128, TT, HL + KVL], F32)
            r_all = persist.tile([128, TT, HL + KVL], F32)

            nc.sync.dma_start(
                wq_sb[:],
                wqT.rearrange("(dt p) j -> p dt j", p=128))
            nc.sync.dma_start(
                wkv_sb[:],
                wkvT.rearrange("(dt p) j -> p dt j", p=128))
            nc.sync.dma_start(
                wo_sb[:],
                woT.rearrange("(ft p) j -> p ft j", p=128))

            # ones columns of v (softmax denominator comes out of the PV
            # matmul)
            nc.gpsimd.memset(v_sb[:, :, HD:HD + 1], 1.0)
            nc.gpsimd.memset(v_sb[:, :, 2 * HD + 1:2 * HD + 2], 1.0)

            # Everything below shares one pool scope: stage 1 (projections /
            # rms / rope / transposes), stage 2 (attention) and stage 3
            # (output projection) are interleaved per q-group so the scalar
            # engine's exp stream — the global bottleneck — starts early and
            # never starves.  PSUM is only 8 banks, so the three stages share
            # tag rings: psA (2 banks x2) carries qkv-proj, score and out-proj
            # accumulators; psB (1 bank x2) carries every 128x128 transpose.
            with (
                tc.tile_pool(name="s1", bufs=2) as s1,
                tc.tile_pool(name="s2", bufs=2) as s2,
                tc.tile_pool(name="psA", bufs=2, space="PSUM") as psA,
                tc.tile_pool(name="psQ", bufs=1, space="PSUM") as psQ,
                tc.tile_pool(name="psB", bufs=1, space="PSUM") as psB,
                tc.tile_pool(name="psY", bufs=1, space="PSUM") as psY,
            ):
                held = {}     # tt -> (x_t, q_sb, k_sb) between sub-stages

                def load_x(tt):
                    _LABELS.append((('load_x', tt), nc.next_id()))
                    x_t = s1.tile([128, DT, 128], F32R, tag="x_t", bufs=4)
                    nc.sync.dma_start(x_t[:], xTr[:, :, 128 * tt:128 * (tt + 1)])
                    tsl = slice(128 * tt, 128 * (tt + 1))
                    cq = s1.tile([128, JQ], F16, tag="cq", bufs=4)
                    nc.sync.dma_start(cq[:], cqi[tsl, :])
                    sq = s1.tile([128, JQ], F16, tag="sq", bufs=4)
                    nc.sync.dma_start(sq[:], sqi[tsl, :])
                    ck = s1.tile([128, JKV], F16, tag="ck", bufs=4)
                    nc.sync.dma_start(ck[:], cki[tsl, :])
                    sk = s1.tile([128, JKV], F16, tag="sk", bufs=4)
                    nc.sync.dma_start(sk[:], ski[tsl, :])
                    held[tt] = [x_t, cq, sq, ck, sk]

                def stage1_a(tt):
                    _LABELS.append((('s1a', tt), nc.next_id()))
                    """Projections + evacuations + rms statistics."""
                    x_t = held[tt][0]
                    ps = psQ.tile([128, 2, 512], F32, tag="psQ", bufs=1)
                    psq, pskv = ps[:, 0, :], ps[:, 1, 0:2 * JKV]
                    for dt in range(DT):
                        nc.tensor.matmul(psq, x_t[:, dt, :], wq_sb[:, dt, :],
                                         start=(dt == 0), stop=(dt == DT - 1))
                    for dt in range(DT):
                        nc.tensor.matmul(pskv, x_t[:, dt, :], wkv_sb[:, dt, :],
                                         start=(dt == 0), stop=(dt == DT - 1))

                    # q_sb/k_sb are held across the 4-tile Newton batch, so
                    # their rings need 4 live buffers
                    q_sb = s1.tile([128, JQ], F16, tag="q_sb", bufs=4)
                    nc.scalar.copy(q_sb[:], psq)
                    k_sb = s1.tile([128, JKV], F16, tag="k_sb", bufs=4)
                    nc.vector.tensor_copy(k_sb[:], pskv[:, 0:JKV])
                    nc.scalar.copy(
                        v_sb[:, tt, :].rearrange("p (u f) -> p u f", u=KVL)
                        [:, :, 0:HD],
                        pskv[:, JKV:2 * JKV].rearrange("p (u f) -> p u f",
                                                       u=KVL))

                    # rms statistics: bn_stats gives [cnt, mean, M2] for the
                    # even and odd halves of each head's 64 features.  Read
                    # the SBUF copies, not the psum, so the psQ ring is freed
                    # by the (fast) evacuations alone.
                    nc.vector.bn_stats(
                        stats[:, tt, 0:HL, :],
                        q_sb.rearrange("p (h f) -> p h f", h=HL))
                    nc.vector.bn_stats(
                        stats[:, tt, HL:HL + KVL, :],
                        k_sb.rearrange("p (h f) -> p h f", h=KVL))
                    held[tt][0] = None
                    held[tt].extend([q_sb, k_sb])

                def newton(k):
                    _LABELS.append((('newton', k), nc.next_id()))
                    """r = (sumsq/64 + eps)^-0.5 for tiles 2k..2k+1.

                    Newton on DVE (seed y0=(a+b*m)/m has <=13% rel err for
                    m in [0.1,2]; 3 iterations -> ~1e-6).
                    """
                    st = stats[:, 2 * k:2 * (k + 1), :].unsqueeze(3)
                    m = s1.tile([128, 2, HL + KVL, 1], F32, tag="nw_m")
                    # m = sumsq/64 + eps
                    nc.vector.tensor_scalar(m[:], st, 1.0 / HD, EPS,
                                            ALU.mult, ALU.add)
                    w = s1.tile([128, 2, HL + KVL, 1], F32, tag="nw_w")
                    nc.vector.reciprocal(w[:], m[:])
                    r = r_all[:, 2 * k:2 * (k + 1), :].unsqueeze(3)
                    nc.vector.tensor_scalar(r, m[:], 0.657, 0.294,
                                            ALU.mult, ALU.add)
                    nc.vector.tensor_mul(r, r, w[:])
                    nt = s1.tile([128, 2, HL + KVL, 1], F32, tag="nw_nt")
                    for _ in range(3):
                        nc.vector.tensor_mul(nt[:], r, r)
                        nc.vector.tensor_mul(nt[:], nt[:], m[:])
                        nc.vector.tensor_scalar(nt[:], nt[:], -0.5, 1.5,
                                                ALU.mult, ALU.add)
                        nc.vector.tensor_mul(r, r, nt[:])

                def s1b_rope(tt):
                    _LABELS.append((('s1b', tt), nc.next_id()))
                    """Apply rms scale and rope (DVE/Pool only)."""
                    _, cq, sq, ck, sk, q_sb, k_sb = held[tt]
                    # scale q rows by their rms factors (Pool; in place)
                    for h in range(HL):
                        nc.gpsimd.tensor_scalar_mul(
                            q_sb[:, 64 * h:64 * (h + 1)],
                            q_sb[:, 64 * h:64 * (h + 1)],
                            r_all[:, tt, h:h + 1])
                    for u in range(KVL):
                        nc.gpsimd.tensor_scalar_mul(
                            k_sb[:, 64 * u:64 * (u + 1)],
                            k_sb[:, 64 * u:64 * (u + 1)],
                            r_all[:, tt, HL + u:HL + u + 1])

                    def rope(dst, src, cos_t, sin_t, tmp, eng):
                        swap = src.rearrange("p (a two) -> p a two", two=2)
                        eng.tensor_mul(
                            tmp.rearrange("p (a two) -> p a two", two=2),
                            swap[:, :, ::-1],
                            sin_t.rearrange("p (a two) -> p a two", two=2))
                        eng.tensor_mul(dst, src, cos_t)
                        eng.tensor_add(dst, dst, tmp)

                    qr = s1.tile([128, JQ], F16, tag="qr", bufs=4)
                    tmpq = s1.tile([128, JQ], F16, tag="tmpq", bufs=4)
                    rope(qr[:], q_sb[:], cq[:], sq[:], tmpq[:], nc.vector)
                    kr = s1.tile([128, JKV], F16, tag="kr", bufs=4)
                    tmpk = s1.tile([128, JKV], F16, tag="tmpk", bufs=4)
                    rope(kr[:], k_sb[:], ck[:], sk[:], tmpk[:], nc.gpsimd)
                    held[tt].extend([qr, kr])

                def s1b_transpose(tt):
                    """Transpose rope output into feature-major qT/kT.

                    q feature block j holds [head j of kv0 | head j+4 of
                    kv1], so transposed partitions align with kT's kv
                    blocks at offsets {0,64}."""
                    qr, kr = held.pop(tt)[-2:]
                    tsl = slice(128 * tt, 128 * (tt + 1))
                    ptr = psB.tile([128, 4, 128], F16, tag="psB")
                    for j in range(4):
                        nc.tensor.transpose(ptr[:, j, :],
                                            qr[:, 128 * j:128 * (j + 1)],
                                            ident[:])
                    nc.vector.tensor_copy(qT_sb[:, :, tsl], ptr[:])
                    ptk = psB.tile([128, 4, 128], F16, tag="psB", name="ptk")
                    nc.tensor.transpose(ptk[:, 0, :], kr[:], ident[:])
                    nc.vector.tensor_copy(kT_sb[:, tsl], ptk[:, 0, :])

                def s1_batch_gen(b):
                    """Full stage 1 for tiles 4b..4b+3, as scheduler quanta.

                    Yields the approximate PE-ns of each emitted quantum so
                    the driver can meter PE filler between QK score tiles.
                    """
                    tiles = range(4 * b, 4 * b + 4)
                    for tt in tiles:
                        load_x(tt)
                        yield 0

                    def s1a(tt):
                        _LABELS.append((('s1a', tt), nc.next_id()))
                        x_t = held[tt][0]
                        # separate single-bank rings: proj-q of tile t+1 only
                        # waits on the q evacuation of tile t
                        psq = psQ.tile([128, 512], F32, tag="psq", bufs=1)
                        pskv = psQ.tile([128, 2 * JKV], F32, tag="pskv",
                                        bufs=1)
                        for half in range(2):
                            for dt in range(4 * half, 4 * half + 4):
                                nc.tensor.matmul(
                                    psq[:], x_t[:, dt, :], wq_sb[:, dt, :],
                                    start=(dt == 0), stop=(dt == DT - 1),
                                    skip_group_check=True)
                            yield 854
                        for half in range(2):
                            for dt in range(4 * half, 4 * half + 4):
                                nc.tensor.matmul(
                                    pskv[:], x_t[:, dt, :], wkv_sb[:, dt, :],
                                    start=(dt == 0), stop=(dt == DT - 1),
                                    skip_group_check=True)
                            yield 427
                        # evacuations (all DVE; ACT stays a pure exp stream)
                        # and rms statistics off the SBUF copies
                        q_sb = s1.tile([128, JQ], F16, tag="q_sb", bufs=4)
                        nc.vector.tensor_copy(q_sb[:], psq[:])
                        k_sb = s1.tile([128, JKV], F16, tag="k_sb", bufs=4)
                        nc.vector.tensor_copy(k_sb[:], pskv[:, 0:JKV])
                        nc.vector.tensor_copy(
                            v_sb[:, tt, :].rearrange("p (u f) -> p u f",
                                                     u=KVL)[:, :, 0:HD],
                            pskv[:, JKV:2 * JKV].rearrange(
                                "p (u f) -> p u f", u=KVL))
                        sq = s1.tile([128, JQ + JKV], F16, tag="sqsc", name="sqsc")
                        nc.vector.tensor_mul(sq[:, 0:JQ], q_sb[:], q_sb[:])
                        nc.vector.tensor_mul(sq[:, JQ:JQ + JKV], k_sb[:],
                                             k_sb[:])
                        nc.vector.reduce_sum(
                            out=stats[:, tt, :].unsqueeze(2),
                            in_=sq.rearrange("p (h f) -> p h f", h=HL + KVL),
                            axis=mybir.AxisListType.X)
                        held[tt][0] = None
                        held[tt].extend([q_sb, k_sb])

                    # 2-tile sub-batches so ropes unblock early: the Newton
                    # rsqrt chain only gates two tiles at a time
                    for half in range(2):
                        sub = list(tiles)[2 * half:2 * half + 2]
                        for tt in sub:
                            for q in s1a(tt):
                                yield q
                            yield 0
                        newton(2 * b + half)
                        yield 0
                        for tt in sub:
                            s1b_rope(tt)
                            s1b_transpose(tt)
                            yield 265

                def qk_gen(g, s, expT):
                    """Scores + exp for head slot s, one psum tile at a time."""
                    _LABELS.append((('qk', g, s), nc.next_id()))
                    u, j = s % 2, s // 2
                    qrhs = qT_sb[64 * u:64 * (u + 1), j,
                                 512 * g:512 * (g + 1)]
                    # full rectangle k-tiles, 2 per psum tile; exp fused into
                    # the ACT evacuation (2 k-tiles per instruction)
                    for c in range(2 * g):
                        pss = psA.tile([128, 2, 512], F32, tag="psA",
                                       name="pss")
                        for lane in range(2):
                            kt = 2 * c + lane
                            nc.tensor.matmul(
                                pss[:, lane, :],
                                kT_sb[64 * u:64 * (u + 1),
                                      128 * kt:128 * (kt + 1)],
                                qrhs)
                        nc.scalar.activation(expT[:, 2 * c:2 * c + 2, :],
                                             pss[:], AFT.Exp,
                                             scale=0.125, bias=bias_m4[:])
                        yield 427
                    # diagonal k-tiles (causal frontier); the sub-diagonal
                    # 128-col block of each is masked on GPSIMD
                    for dc in range(2):
                        pss = psA.tile([128, 2, 512], F32, tag="psA",
                                       name="pss")
                        for lane in range(2):
                            kt = 4 * g + 2 * dc + lane
                            n0 = 128 * (2 * dc + lane)
                            nc.tensor.matmul(
                                pss[:, lane, n0:512],
                                kT_sb[64 * u:64 * (u + 1),
                                      128 * kt:128 * (kt + 1)],
                                qrhs[:, n0:512])
                            nc.scalar.activation(expT[:, kt, n0:512],
                                                 pss[:, lane, n0:512],
                                                 AFT.Exp,
                                                 scale=0.125, bias=bias_m4[:])
                            nc.gpsimd.tensor_mul(expT[:, kt, n0:n0 + 128],
                                                 expT[:, kt, n0:n0 + 128],
                                                 mask01[:])
                        yield 374

                def pv_gen(g, s, expT, y_sb):
                    """PV + softmax normalization for head slot s."""
                    _LABELS.append((('pv', g, s), nc.next_id()))
                    u = s % 2
                    psy = psY.tile([128, 4, HD + 1], F32, tag="psy")
                    for i in range(4):
                        nkt = 4 * g + i + 1
                        for kt in range(nkt):
                            nc.tensor.matmul(
                                psy[:, i, :],
                                expT[:, kt, 128 * i:128 * (i + 1)],
                                v_sb[:, kt, (HD + 1) * u:(HD + 1) * (u + 1)],
                                start=(kt == 0), stop=(kt == nkt - 1))
                        yield 27 * nkt
                    # one fast evacuation frees the psY bank; normalize
                    # afterwards from SBUF (4x-mode TSPs)
                    y_un = s2.tile([128, 4, HD + 1], F16, tag="y_un")
                    nc.vector.tensor_copy(y_un[:], psy[:])
                    rl = s2.tile([128, 4, 1], F32, tag="rl")
                    nc.vector.reciprocal(rl[:], y_un[:, :, HD:HD + 1])
                    for i in range(4):
                        nc.vector.tensor_scalar_mul(
                            y_sb[:, i, 64 * s:64 * (s + 1)],
                            y_un[:, i, 0:HD], rl[:, i, :])
                    yield 0

                def s3_gen(g, y_sb):
                    """Output projection for q-group g (4 row blocks)."""
                    for i in range(4):
                        _LABELS.append((('s3', g, i), nc.next_id()))
                        ptt = psB.tile([128, 4, 128], F16, tag="psB",
                                       name="ptt")
                        for ft in range(4):
                            nc.tensor.transpose(
                                ptt[:, ft, :],
                                y_sb[:, i, 128 * ft:128 * (ft + 1)],
                                ident[:])
                        yT = s2.tile([128, 4, 128], F16, tag="yT")
                        nc.vector.tensor_copy(yT[:], ptt[:])
                        yield 212
                        pso = psA.tile([128, 2, 512], F32, tag="psA",
                                       name="pso")
                        for nt in range(2):
                            for ft in range(4):
                                nc.tensor.matmul(
                                    pso[:, nt, :], yT[:, ft, :],
                                    wo_sb[:, ft, 512 * nt:512 * (nt + 1)],
                                    start=(ft == 0), stop=(ft == 3))
                            yield 854
                        out_sb = s2.tile([128, D], F32, tag="out_sb")
                        nc.vector.tensor_copy(out_sb[:], pso[:])
                        r0 = 512 * g + 128 * i
                        nc.sync.dma_start(outp[r0:r0 + 128, :], out_sb[:])
                        yield 0

                # ---- driver: emit QK score tiles (the ACT pacers) round-
                # robined with metered PE filler from the deferred queues.
                from collections import deque
                bulk = deque()     # stage-1 batches and stage-3 groups
                prio = deque()     # PV generators (free the expT ring)

                s1_gens = {}

                def drain(gen):
                    for _ in gen:
                        pass

                def pump(target):
                    got = 0
                    while got < target and (bulk or prio):
                        q = bulk[0] if bulk else prio[0]
                        try:
                            got += next(q)
                        except StopIteration:
                            if bulk and q is bulk[0]:
                                bulk.popleft()
                            else:
                                prio.popleft()
                    return got

                # prologue: stage 1 for tiles 0-3 (q-group 0's span);
                # later batches are queued up front and pumped as filler
                drain(s1_batch_gen(0))
                for b in range(1, 4):
                    s1_gens[b] = s1_batch_gen(b)
                    bulk.append(s1_gens[b])

                ys = {}
                pv_gens = {}
                for hi, (g, s) in enumerate(
                        (g, s) for g in range(G) for s in range(HL)):
                    if s == 0:
                        # tiles 4g..4g+3 must be fully emitted before this
                        # group's QKs reference qT/kT (emission order is
                        # engine program order)
                        if g in s1_gens:
                            drain(s1_gens.pop(g))
                        # stage 3 is deferred into later, ACT-heavier groups
                        if g == 2:
                            bulk.append(s3_gen(0, ys[0]))
                        if g == 3:
                            bulk.append(s3_gen(1, ys[1]))
                            bulk.append(s3_gen(2, ys[2]))
                        ys[g] = s2.tile([128, 4, JQ], F16, tag="y_sb",
                                        bufs=4, name="y_sb")
                    # the expT ring is 2 deep: pv(hi-2) must be fully
                    # emitted before expT[hi] is allocated over its slot
                    if hi - 2 in pv_gens:
                        drain(pv_gens.pop(hi - 2))
                    expT_h = s2.tile([128, 4 * g + 4, 512], F16,
                                     tag="expT", name="expT")
                    for cost in qk_gen(g, s, expT_h):
                        pump(int(cost * 1.0))
                    pv_gens[hi] = pv_gen(g, s, expT_h, ys[g])
                    prio.append(pv_gens[hi])
                # tail: remaining PV, deferred work, then the last stage 3
                for k in sorted(pv_gens):
                    drain(pv_gens.pop(k))
                while prio or bulk:
                    pump(1 << 30)
                drain(s3_gen(3, ys[3]))

    nc.compile()
    return nc


_PROGRAM_CACHE = {}
_LABELS = []

# within-head feature interleave: slot 2m <- feat m, slot 2m+1 <- feat 32+m
IVF = np.empty(HD, dtype=np.int64)
IVF[0::2] = np.arange(32)
IVF[1::2] = np.arange(32, 64)

# q-head slot order: feature block j holds heads (j, j+4) = (j of kv0,
# j of kv1); y slot s holds head (s//2) + 4*(s%2)
QBLK = [0, 4, 1, 5, 2, 6, 3, 7]      # feature order for Wq cols / rope
YSLOT = [0, 4, 1, 5, 2, 6, 3, 7]     # y_sb slot s -> local head


def _rope_tables(n_heads, gains):
    """Pair-interleaved cos/sin tables [S, n_heads*64] with the rotation
    sign folded into sin: slot 2m gets (cos, sin), slot 2m+1 (cos, -sin)."""
    inv_freq = 1.0 / (ROPE_BASE ** (np.arange(0, HD, 2, dtype=np.float32) / HD))
    t = np.arange(S, dtype=np.float32)
    freqs = np.outer(t, inv_freq)                    # [S, 32]
    cos, sin = np.cos(freqs), np.sin(freqs)
    ct = np.empty((S, n_heads, HD), dtype=np.float32)
    st = np.empty((S, n_heads, HD), dtype=np.float32)
    for h in range(n_heads):
        g = gains[h]
        ct[:, h, 0::2] = cos * g
        ct[:, h, 1::2] = cos * g
        st[:, h, 0::2] = sin * g
        st[:, h, 1::2] = -sin * g
    return (np.ascontiguousarray(ct.reshape(S, n_heads * HD), dtype=np.float16),
            np.ascontiguousarray(st.reshape(S, n_heads * HD), dtype=np.float16))


def _in_map_for_core(x, Wq, Wk, Wv, Wo, q_gain, core):
    b, hh = core // 2, core % 2
    lq0 = HL * hh                         # first local q head (global index)
    kvh = slice(JKV * hh, JKV * (hh + 1))

    # Wq rows in (block j: head j, head j+4) order, pair-interleaved feats
    qrows = np.concatenate([64 * (lq0 + h) + IVF for h in QBLK])
    # Wk rows pair-interleaved per kv head; Wv rows plain
    krows = np.concatenate([64 * u + IVF for u in range(KVL)])
    wkv = np.concatenate([Wk[kvh, :][krows, :], Wv[kvh, :]], axis=0)
    # Wo cols for y slot order
    orows = np.concatenate([64 * (lq0 + h) + np.arange(64) for h in YSLOT])

    gains = q_gain[[lq0 + h for h in QBLK]]
    cq, sq = _rope_tables(HL, gains)
    ck, sk = _rope_tables(KVL, np.ones(KVL, dtype=np.float32))
    return {
        "xT": np.ascontiguousarray(x[b].T),
        "wqT": np.ascontiguousarray(Wq[qrows, :].T),
        "wkvT": np.ascontiguousarray(wkv.T),
        "woT": np.ascontiguousarray(Wo[:, orows].T.astype(np.float16)),
        "cqi": cq, "sqi": sq, "cki": ck, "ski": sk,
    }


def kernel(x, Wq, Wk, Wv, Wo, q_gain):
    x = np.asarray(x, dtype=np.float32)
    Wq = np.asarray(Wq, dtype=np.float32)
    Wk = np.asarray(Wk, dtype=np.float32)
    Wv = np.asarray(Wv, dtype=np.float32)
    Wo = np.asarray(Wo, dtype=np.float32)
    q_gain = np.asarray(q_gain, dtype=np.float32)

    if "nc" not in _PROGRAM_CACHE:
        _PROGRAM_CACHE["nc"] = _build_program()
    nc = _PROGRAM_CACHE["nc"]

    in_maps = [_in_map_for_core(x, Wq, Wk, Wv, Wo, q_gain, core)
               for core in range(N_CORES)]

    res = run_bass_kernel_spmd(nc, in_maps, core_ids=list(range(N_CORES)))
    _PROGRAM_CACHE["last_results"] = res

    out = np.empty((B, S, D), dtype=np.float32)
    for b in range(B):
        out[b] = res.results[2 * b]["outp"] + res.results[2 * b + 1]["outp"]
    return out


if __name__ == "__main__":
    rng = np.random.default_rng(0)
    inputs = {
        "x": rng.standard_normal((B, S, D), dtype=np.float32),
        "Wq": rng.standard_normal((D, D), dtype=np.float32) * 0.02,
        "Wk": rng.standard_normal((KVH * HD, D), dtype=np.float32) * 0.02,
        "Wv": rng.standard_normal((KVH * HD, D), dtype=np.float32) * 0.02,
        "Wo": rng.standard_normal((D, D), dtype=np.float32) * 0.02,
        "q_gain": np.full((H,), 1.5, dtype=np.float32),
    }
    out = kernel(**inputs)
    print(out.shape, out.dtype, np.abs(out).max())


# revision 44
# speedup vs baseline: 1.2531x; 1.0518x over previous
"""Trainium2 Bass kernel for a dense transformer attention block.

Reference computation (fp32):
  q = rms_norm(x @ Wq.T)  per head (16 heads x 64)  -> rope -> * q_gain
  k = rms_norm(x @ Wk.T)  per kv-head (4 x 64)      -> rope
  v = x @ Wv.T
  causal GQA attention (16 q heads over 4 kv heads), softmax(q k / 8)
  out = (attn @ v) @ Wo.T

Sharding over 8 cores: core c = 2*b + hh handles batch b (of 4) and
q-head half hh (8 q heads = 2 kv heads).  Each core produces a partial
out [2048, 1024] (its heads' contribution through Wo); the host adds
the two partials per batch.  No collectives.

Datapath design (tuned against the TimelineSim cost model):
  - projections run in fp32r (full speed at free-size>=256); everything
    downstream (q/k/v, rope tables, probabilities, y, Wo) is fp16, which
    doubles DVE throughput (2x_1p) and keeps all matmuls at 1 cycle/row
  - q/k features are pair-interleaved (slot 2m <- feat m, 2m+1 <- feat
    32+m) so rope is 3 wide DVE ops using a stride -1 pair-swap view;
    the rotation sign lives in the host-built sin table; q_gain is
    folded into the q cos/sin tables
  - RMS factors: fp16 square + one grouped reduce per tile, then a
    Newton rsqrt batched over 2 token tiles (~10 small DVE ops)
  - scores are built transposed ([k, q]) so softmax needs no transposes:
    exp(s/8 - 4) is fused into the ACT evacuation of the QK psum (2
    k-tiles per instruction), the denominator comes from a ones column
    in v, and the normalization is applied per-partition post-PV
  - causal frontier masking and the rms scale/rope of k run on the
    otherwise-idle GPSIMD engine

Scheduling: all three stages are emitted through a quantum scheduler —
QK score tiles (the pacers of the scalar engine's ~157us exp stream)
alternate with metered filler (PV of earlier heads, stage-1 batches for
later q-groups, deferred output projections), so the exp stream starts
early and the 8 PSUM banks stay within budget via shared tag rings.
"""

import hashlib
import os

import numpy as np

# The libneuronxla NEFF cache can key-collide across different kernel
# versions with identical I/O shapes (observed: a stale NEFF served for an
# edited kernel).  Key the cache by this file's content so a changed kernel
# never hits a stale entry while identical re-runs stay warm.
try:
    _SRC_HASH = hashlib.sha256(open(__file__, "rb").read()).hexdigest()[:16]
except OSError:
    _SRC_HASH = "nosrc"
os.environ["NEURON_COMPILE_CACHE_URL"] = os.path.join(
    os.environ.get("TMPDIR", "/tmp"), f"neuron-cache-{_SRC_HASH}")

import concourse.bass as bass
import concourse.mybir as mybir
import concourse.tile as tile
from concourse import bacc
from concourse.bass_utils import run_bass_kernel_spmd
from concourse.masks import make_identity, make_upper_triangular

F32 = mybir.dt.float32
F32R = mybir.dt.float32r
F16 = mybir.dt.float16
AFT = mybir.ActivationFunctionType
ALU = mybir.AluOpType

B, S, D = 4, 2048, 1024
H, HD, KVH = 16, 64, 4
HL = 8            # q heads per core
KVL = 2           # kv heads per core
JQ = HL * HD      # 512 q-proj cols per core
JKV = KVL * HD    # 128 k (or v) proj cols per core
TT = S // 128     # 16 token tiles
DT = D // 128     # 8 contraction tiles
G = 4             # q groups of 512
ROPE_BASE = 10000.0
EPS = 1e-6
N_CORES = 8


def _build_program():
    nc = bacc.Bacc("TRN2", target_bir_lowering=False, debug=False,
                   num_devices=N_CORES)

    xT = nc.dram_tensor("xT", [D, S], F32R, kind="ExternalInput").ap()
    wqT = nc.dram_tensor("wqT", [D, JQ], F32R, kind="ExternalInput").ap()
    wkvT = nc.dram_tensor("wkvT", [D, 2 * JKV], F32R, kind="ExternalInput").ap()
    woT = nc.dram_tensor("woT", [JQ, D], F16, kind="ExternalInput").ap()
    cqi = nc.dram_tensor("cqi", [S, JQ], F16, kind="ExternalInput").ap()
    sqi = nc.dram_tensor("sqi", [S, JQ], F16, kind="ExternalInput").ap()
    cki = nc.dram_tensor("cki", [S, JKV], F16, kind="ExternalInput").ap()
    ski = nc.dram_tensor("ski", [S, JKV], F16, kind="ExternalInput").ap()
    outp = nc.dram_tensor("outp", [S, D], F32, kind="ExternalOutput").ap()
    xTr = xT.rearrange("(dt p) s -> p dt s", p=128)

    with tile.TileContext(nc) as tc:
        with (
            tc.tile_pool(name="consts", bufs=1) as consts,
            tc.tile_pool(name="persist", bufs=1) as persist,
        ):
            ident = consts.tile([128, 128], F16)
            make_identity(nc, ident)
            mask01 = consts.tile([128, 128], F16)
            make_upper_triangular(nc, mask01, val=1.0, diag=True)
            bias_m4 = consts.tile([128, 1], F32)
            nc.gpsimd.memset(bias_m4[:], -4.0)

            wq_sb = persist.tile([128, DT, JQ], F32R)
            wkv_sb = persist.tile([128, DT, 2 * JKV], F32R)
            wo_sb = persist.tile([128, JQ // 128, D], F16)
            qT_sb = persist.tile([128, 4, S], F16)
            kT_sb = persist.tile([128, S], F16)
            v_sb = persist.tile([128, TT, 2 * (HD + 1)], F16)
            stats = persist.tile([128, TT, HL + KVL], F32)
            r_all = persist.tile([128, TT, HL + KVL], F32)

            nc.sync.dma_start(
                wq_sb[:],
                wqT.rearrange("(dt p) j -> p dt j", p=128))
            nc.sync.dma_start(
                wkv_sb[:],
                wkvT.rearrange("(dt p) j -> p dt j", p=128))
            nc.sync.dma_start(
                wo_sb[:],
                woT.rearrange("(ft p) j -> p ft j", p=128))

            # ones columns of v (softmax denominator comes out of the PV
            # matmul)
            nc.gpsimd.memset(v_sb[:, :, HD:HD + 1], 1.0)
            nc.gpsimd.memset(v_sb[:, :, 2 * HD + 1:2 * HD + 2], 1.0)

            # Everything below shares one pool scope: stage 1 (projections /
            # rms / rope / transposes), stage 2 (attention) and stage 3
            # (output projection) are interleaved per q-group so the scalar
            # engine's exp stream — the global bottleneck — starts early and
            # never starves.  PSUM is only 8 banks, so the three stages share
            # tag rings: psA (2 banks x2) carries qkv-proj, score and out-proj
            # accumulators; psB (1 bank x2) carries every 128x128 transpose.
            with (
                tc.tile_pool(name="s1", bufs=2) as s1,
                tc.tile_pool(name="s2", bufs=2) as s2,
                tc.tile_pool(name="psA", bufs=2, space="PSUM") as psA,
                tc.tile_pool(name="psQ", bufs=1, space="PSUM") as psQ,
                tc.tile_pool(name="psB", bufs=1, space="PSUM") as psB,
                tc.tile_pool(name="psY", bufs=1, space="PSUM") as psY,
            ):
                held = {}     # tt -> (x_t, q_sb, k_sb) between sub-stages

                def load_x(tt):
                    _LABELS.append((('load_x', tt), nc.next_id()))
                    x_t = s1.tile([128, DT, 128], F32R, tag="x_t", bufs=4)
                    nc.sync.dma_start(x_t[:], xTr[:, :, 128 * tt:128 * (tt + 1)])
                    tsl = slice(128 * tt, 128 * (tt + 1))
                    cq = s1.tile([128, JQ], F16, tag="cq", bufs=4)
                    nc.sync.dma_start(cq[:], cqi[tsl, :])
                    sq = s1.tile([128, JQ], F16, tag="sq", bufs=4)
                    nc.sync.dma_start(sq[:], sqi[tsl, :])
                    ck = s1.tile([128, JKV], F16, tag="ck", bufs=4)
                    nc.sync.dma_start(ck[:], cki[tsl, :])
                    sk = s1.tile([128, JKV], F16, tag="sk", bufs=4)
                    nc.sync.dma_start(sk[:], ski[tsl, :])
                    held[tt] = [x_t, cq, sq, ck, sk]

                def stage1_a(tt):
                    _LABELS.append((('s1a', tt), nc.next_id()))
                    """Projections + evacuations + rms statistics."""
                    x_t = held[tt][0]
                    ps = psQ.tile([128, 2, 512], F32, tag="psQ", bufs=1)
                    psq, pskv = ps[:, 0, :], ps[:, 1, 0:2 * JKV]
                    for dt in range(DT):
                        nc.tensor.matmul(psq, x_t[:, dt, :], wq_sb[:, dt, :],
                                         start=(dt == 0), stop=(dt == DT - 1))
                    for dt in range(DT):
                        nc.tensor.matmul(pskv, x_t[:, dt, :], wkv_sb[:, dt, :],
                                         start=(dt == 0), stop=(dt == DT - 1))

                    # q_sb/k_sb are held across the 4-tile Newton batch, so
                    # their rings need 4 live buffers
                    q_sb = s1.tile([128, JQ], F16, tag="q_sb", bufs=4)
                    nc.scalar.copy(q_sb[:], psq)
                    k_sb = s1.tile([128, JKV], F16, tag="k_sb", bufs=4)
                    nc.vector.tensor_copy(k_sb[:], pskv[:, 0:JKV])
                    nc.scalar.copy(
                        v_sb[:, tt, :].rearrange("p (u f) -> p u f", u=KVL)
                        [:, :, 0:HD],
                        pskv[:, JKV:2 * JKV].rearrange("p (u f) -> p u f",
                                                       u=KVL))

                    # rms statistics: bn_stats gives [cnt, mean, M2] for the
                    # even and odd halves of each head's 64 features.  Read
                    # the SBUF copies, not the psum, so the psQ ring is freed
                    # by the (fast) evacuations alone.
                    nc.vector.bn_stats(
                        stats[:, tt, 0:HL, :],
                        q_sb.rearrange("p (h f) -> p h f", h=HL))
                    nc.vector.bn_stats(
                        stats[:, tt, HL:HL + KVL, :],
                        k_sb.rearrange("p (h f) -> p h f", h=KVL))
                    held[tt][0] = None
                    held[tt].extend([q_sb, k_sb])

                def newton(k):
                    _LABELS.append((('newton', k), nc.next_id()))
                    """r = (sumsq/64 + eps)^-0.5 for tiles 2k..2k+1.

                    Newton on DVE (seed y0=(a+b*m)/m has <=13% rel err for
                    m in [0.1,2]; 3 iterations -> ~1e-6).
                    """
                    st = stats[:, 2 * k:2 * (k + 1), :].unsqueeze(3)
                    m = s1.tile([128, 2, HL + KVL, 1], F32, tag="nw_m")
                    # m = sumsq/64 + eps
                    nc.vector.tensor_scalar(m[:], st, 1.0 / HD, EPS,
                                            ALU.mult, ALU.add)
                    w = s1.tile([128, 2, HL + KVL, 1], F32, tag="nw_w")
                    nc.vector.reciprocal(w[:], m[:])
                    r = r_all[:, 2 * k:2 * (k + 1), :].unsqueeze(3)
                    nc.vector.tensor_scalar(r, m[:], 0.657, 0.294,
                                            ALU.mult, ALU.add)
                    nc.vector.tensor_mul(r, r, w[:])
                    nt = s1.tile([128, 2, HL + KVL, 1], F32, tag="nw_nt")
                    for _ in range(3):
                        nc.vector.tensor_mul(nt[:], r, r)
                        nc.vector.tensor_mul(nt[:], nt[:], m[:])
                        nc.vector.tensor_scalar(nt[:], nt[:], -0.5, 1.5,
                                                ALU.mult, ALU.add)
                        nc.vector.tensor_mul(r, r, nt[:])

                def s1b_rope(tt):
                    _LABELS.append((('s1b', tt), nc.next_id()))
                    """Apply rms scale and rope (DVE/Pool only)."""
                    _, cq, sq, ck, sk, q_sb, k_sb = held[tt]
                    # scale q rows by their rms factors (Pool; in place)
                    for h in range(HL):
                        nc.gpsimd.tensor_scalar_mul(
                            q_sb[:, 64 * h:64 * (h + 1)],
                            q_sb[:, 64 * h:64 * (h + 1)],
                            r_all[:, tt, h:h + 1])
                    for u in range(KVL):
                        nc.gpsimd.tensor_scalar_mul(
                            k_sb[:, 64 * u:64 * (u + 1)],
                            k_sb[:, 64 * u:64 * (u + 1)],
                            r_all[:, tt, HL + u:HL + u + 1])

                    def rope(dst, src, cos_t, sin_t, tmp, eng):
                        swap = src.rearrange("p (a two) -> p a two", two=2)
                        eng.tensor_mul(
                            tmp.rearrange("p (a two) -> p a two", two=2),
                            swap[:, :, ::-1],
                            sin_t.rearrange("p (a two) -> p a two", two=2))
                        eng.tensor_mul(dst, src, cos_t)
                        eng.tensor_add(dst, dst, tmp)

                    qr = s1.tile([128, JQ], F16, tag="qr", bufs=4)
                    tmpq = s1.tile([128, JQ], F16, tag="tmpq", bufs=4)
                    rope(qr[:], q_sb[:], cq[:], sq[:], tmpq[:], nc.vector)
                    kr = s1.tile([128, JKV], F16, tag="kr", bufs=4)
                    tmpk = s1.tile([128, JKV], F16, tag="tmpk", bufs=4)
                    rope(kr[:], k_sb[:], ck[:], sk[:], tmpk[:], nc.gpsimd)
                    held[tt].extend([qr, kr])

                def s1b_transpose(tt):
                    """Transpose rope output into feature-major qT/kT.

                    q feature block j holds [head j of kv0 | head j+4 of
                    kv1], so transposed partitions align with kT's kv
                    blocks at offsets {0,64}."""
                    qr, kr = held.pop(tt)[-2:]
                    tsl = slice(128 * tt, 128 * (tt + 1))
                    ptr = psB.tile([128, 4, 128], F16, tag="psB")
                    for j in range(4):
                        nc.tensor.transpose(ptr[:, j, :],
                                            qr[:, 128 * j:128 * (j + 1)],
                                            ident[:])
                    nc.vector.tensor_copy(qT_sb[:, :, tsl], ptr[:])
                    ptk = psB.tile([128, 4, 128], F16, tag="psB", name="ptk")
                    nc.tensor.transpose(ptk[:, 0, :], kr[:], ident[:])
                    nc.vector.tensor_copy(kT_sb[:, tsl], ptk[:, 0, :])

                def s1_batch_gen(b):
                    """Full stage 1 for tiles 4b..4b+3, as scheduler quanta.

                    Yields the approximate PE-ns of each emitted quantum so
                    the driver can meter PE filler between QK score tiles.
                    """
                    tiles = range(4 * b, 4 * b + 4)
                    for tt in tiles:
                        load_x(tt)
                        yield 0

                    def s1a(tt):
                        _LABELS.append((('s1a', tt), nc.next_id()))
                        x_t = held[tt][0]
                        # separate single-bank rings: proj-q of tile t+1 only
                        # waits on the q evacuation of tile t
                        psq = psQ.tile([128, 512], F32, tag="psq", bufs=1)
                        pskv = psQ.tile([128, 2 * JKV], F32, tag="pskv",
                                        bufs=1)
                        for half in range(2):
                            for dt in range(4 * half, 4 * half + 4):
                                nc.tensor.matmul(
                                    psq[:], x_t[:, dt, :], wq_sb[:, dt, :],
                                    start=(dt == 0), stop=(dt == DT - 1),
                                    skip_group_check=True)
                            yield 854
                        for half in range(2):
                            for dt in range(4 * half, 4 * half + 4):
                                nc.tensor.matmul(
                                    pskv[:], x_t[:, dt, :], wkv_sb[:, dt, :],
                                    start=(dt == 0), stop=(dt == DT - 1),
                                    skip_group_check=True)
                            yield 427
                        # evacuations (all DVE; ACT stays a pure exp stream)
                        # and rms statistics off the SBUF copies
                        q_sb = s1.tile([128, JQ], F16, tag="q_sb", bufs=4)
                        nc.vector.tensor_copy(q_sb[:], psq[:])
                        k_sb = s1.tile([128, JKV], F16, tag="k_sb", bufs=4)
                        nc.scalar.copy(k_sb[:], pskv[:, 0:JKV])
                        nc.scalar.copy(
                            v_sb[:, tt, :].rearrange("p (u f) -> p u f",
                                                     u=KVL)[:, :, 0:HD],
                            pskv[:, JKV:2 * JKV].rearrange(
                                "p (u f) -> p u f", u=KVL))
                        sq = s1.tile([128, JQ + JKV], F32, tag="sqsc", name="sqsc")
                        nc.scalar.activation(sq[:, 0:JQ], psq[:], AFT.Square)
                        nc.scalar.activation(sq[:, JQ:JQ + JKV],
                                             pskv[:, 0:JKV], AFT.Square)
                        nc.vector.reduce_sum(
                            out=stats[:, tt, :].unsqueeze(2),
                            in_=sq.rearrange("p (h f) -> p h f", h=HL + KVL),
                            axis=mybir.AxisListType.X)
                        held[tt][0] = None
                        held[tt].extend([q_sb, k_sb])

                    # 2-tile sub-batches so ropes unblock early: the Newton
                    # rsqrt chain only gates two tiles at a time
                    for half in range(2):
                        sub = list(tiles)[2 * half:2 * half + 2]
                        for tt in sub:
                            for q in s1a(tt):
                                yield q
                            yield 0
                        newton(2 * b + half)
                        yield 0
                        for tt in sub:
                            s1b_rope(tt)
                            s1b_transpose(tt)
                            yield 265

                def qk_gen(g, s, expT):
                    """Scores + exp for head slot s, one psum tile at a time."""
                    _LABELS.append((('qk', g, s), nc.next_id()))
                    u, j = s % 2, s // 2
                    qrhs = qT_sb[64 * u:64 * (u + 1), j,
                                 512 * g:512 * (g + 1)]
                    # full rectangle k-tiles, 2 per psum tile; exp fused into
                    # the ACT evacuation (2 k-tiles per instruction)
                    for c in range(2 * g):
                        pss = psA.tile([128, 2, 512], F32, tag="psA",
                                       name="pss")
                        for lane in range(2):
                            kt = 2 * c + lane
                            nc.tensor.matmul(
                                pss[:, lane, :],
                                kT_sb[64 * u:64 * (u + 1),
                                      128 * kt:128 * (kt + 1)],
                                qrhs)
                        nc.scalar.activation(expT[:, 2 * c:2 * c + 2, :],
                                             pss[:], AFT.Exp,
                                             scale=0.125, bias=bias_m4[:])
                        yield 427
                    # diagonal k-tiles (causal frontier); the sub-diagonal
                    # 128-col block of each is masked on GPSIMD
                    for dc in range(2):
                        pss = psA.tile([128, 2, 512], F32, tag="psA",
                                       name="pss")
                        for lane in range(2):
                            kt = 4 * g + 2 * dc + lane
                            n0 = 128 * (2 * dc + lane)
                            nc.tensor.matmul(
                                pss[:, lane, n0:512],
                                kT_sb[64 * u:64 * (u + 1),
                                      128 * kt:128 * (kt + 1)],
                                qrhs[:, n0:512])
                            nc.scalar.activation(expT[:, kt, n0:512],
                                                 pss[:, lane, n0:512],
                                                 AFT.Exp,
                                                 scale=0.125, bias=bias_m4[:])
                            nc.gpsimd.tensor_mul(expT[:, kt, n0:n0 + 128],
                                                 expT[:, kt, n0:n0 + 128],
                                                 mask01[:])
                        yield 374

                def pv_gen(g, s, expT, y_sb):
                    """PV + softmax normalization for head slot s."""
                    _LABELS.append((('pv', g, s), nc.next_id()))
                    u = s % 2
                    psy = psY.tile([128, 4, HD + 1], F32, tag="psy")
                    for i in range(4):
                        nkt = 4 * g + i + 1
                        for kt in range(nkt):
                            nc.tensor.matmul(
                                psy[:, i, :],
                                expT[:, kt, 128 * i:128 * (i + 1)],
                                v_sb[:, kt, (HD + 1) * u:(HD + 1) * (u + 1)],
                                start=(kt == 0), stop=(kt == nkt - 1))
                        yield 27 * nkt
                    # one fast evacuation frees the psY bank; normalize
                    # afterwards from SBUF (4x-mode TSPs)
                    y_un = s2.tile([128, 4, HD + 1], F16, tag="y_un")
                    nc.vector.tensor_copy(y_un[:], psy[:])
                    rl = s2.tile([128, 4, 1], F32, tag="rl")
                    nc.vector.reciprocal(rl[:], y_un[:, :, HD:HD + 1])
                    for i in range(4):
                        nc.vector.tensor_scalar_mul(
                            y_sb[:, i, 64 * s:64 * (s + 1)],
                            y_un[:, i, 0:HD], rl[:, i, :])
                    yield 0

                def s3_gen(g, y_sb):
                    """Output projection for q-group g (4 row blocks)."""
                    for i in range(4):
                        _LABELS.append((('s3', g, i), nc.next_id()))
                        ptt = psB.tile([128, 4, 128], F16, tag="psB",
                                       name="ptt")
                        for ft in range(4):
                            nc.tensor.transpose(
                                ptt[:, ft, :],
                                y_sb[:, i, 128 * ft:128 * (ft + 1)],
                                ident[:])
                        yT = s2.tile([128, 4, 128], F16, tag="yT")
                        nc.vector.tensor_copy(yT[:], ptt[:])
                        yield 212
                        pso = psA.tile([128, 2, 512], F32, tag="psA",
                                       name="pso")
                        for nt in range(2):
                            for ft in range(4):
                                nc.tensor.matmul(
                                    pso[:, nt, :], yT[:, ft, :],
                                    wo_sb[:, ft, 512 * nt:512 * (nt + 1)],
                                    start=(ft == 0), stop=(ft == 3))
                            yield 854
                        out_sb = s2.tile([128, D], F32, tag="out_sb")
                        nc.vector.tensor_copy(out_sb[:], pso[:])
                        r0 = 512 * g + 128 * i
                        nc.sync.dma_start(outp[r0:r0 + 128, :], out_sb[:])
                        yield 0

                # ---- driver: emit QK score tiles (the ACT pacers) round-
                # robined with metered PE filler from the deferred queues.
                from collections import deque
                bulk = deque()     # stage-1 batches and stage-3 groups
                prio = deque()     # PV generators (free the expT ring)

                s1_gens = {}

                def drain(gen):
                    for _ in gen:
                        pass

                def pump(target):
                    got = 0
                    while got < target and (prio or bulk):
                        q = prio[0] if prio else bulk[0]
                        try:
                            got += next(q)
                        except StopIteration:
                            if prio and q is prio[0]:
                                prio.popleft()
                            else:
                                bulk.popleft()
                    return got

                # prologue: stage 1 for tiles 0-3 (q-group 0's span);
                # later batches are queued up front and pumped as filler
                drain(s1_batch_gen(0))
                for b in range(1, 4):
                    s1_gens[b] = s1_batch_gen(b)
                    bulk.append(s1_gens[b])

                ys = {}
                pv_gens = {}
                for hi, (g, s) in enumerate(
                        (g, s) for g in range(G) for s in range(HL)):
                    if s == 0:
                        # tiles 4g..4g+3 must be fully emitted before this
                        # group's QKs reference qT/kT (emission order is
                        # engine program order)
                        if g in s1_gens:
                            drain(s1_gens.pop(g))
                        # stage 3 is deferred into later, ACT-heavier groups
                        if g == 2:
                            bulk.append(s3_gen(0, ys[0]))
                        if g == 3:
                            bulk.append(s3_gen(1, ys[1]))
                            bulk.append(s3_gen(2, ys[2]))
                        ys[g] = s2.tile([128, 4, JQ], F16, tag="y_sb",
                                        bufs=4, name="y_sb")
                    # the expT ring is 2 deep: pv(hi-2) must be fully
                    # emitted before expT[hi] is allocated over its slot
                    if hi - 2 in pv_gens:
                        drain(pv_gens.pop(hi - 2))
                    expT_h = s2.tile([128, 4 * g + 4, 512], F16,
                                     tag="expT", name="expT")
                    for cost in qk_gen(g, s, expT_h):
                        pump(int(cost * 1.0))
                    pv_gens[hi] = pv_gen(g, s, expT_h, ys[g])
                    prio.append(pv_gens[hi])
                # tail: remaining PV, deferred work, then the last stage 3
                for k in sorted(pv_gens):
                    drain(pv_gens.pop(k))
                while prio or bulk:
                    pump(1 << 30)
                drain(s3_gen(3, ys[3]))

    nc.compile()
    return nc


_PROGRAM_CACHE = {}
_LABELS = []

# within-head feature interleave: slot 2m <- feat m, slot 2m+1 <- feat 32+m
IVF = np.empty(HD, dtype=np.int64)
IVF[0::2] = np.arange(32)
IVF[1::2] = np.arange(32, 64)

# q-head slot order: feature block j holds heads (j, j+4) = (j of kv0,
# j of kv1); y slot s holds head (s//2) + 4*(s%2)
QBLK = [0, 4, 1, 5, 2, 6, 3, 7]      # feature order for Wq cols / rope
YSLOT = [0, 4, 1, 5, 2, 6, 3, 7]     # y_sb slot s -> local head


def _rope_tables(n_heads, gains):
    """Pair-interleaved cos/sin tables [S, n_heads*64] with the rotation
    sign folded into sin: slot 2m gets (cos, sin), slot 2m+1 (cos, -sin)."""
    inv_freq = 1.0 / (ROPE_BASE ** (np.arange(0, HD, 2, dtype=np.float32) / HD))
    t = np.arange(S, dtype=np.float32)
    freqs = np.outer(t, inv_freq)                    # [S, 32]
    cos, sin = np.cos(freqs), np.sin(freqs)
    ct = np.empty((S, n_heads, HD), dtype=np.float32)
    st = np.empty((S, n_heads, HD), dtype=np.float32)
    for h in range(n_heads):
        g = gains[h]
        ct[:, h, 0::2] = cos * g
        ct[:, h, 1::2] = cos * g
        st[:, h, 0::2] = sin * g
        st[:, h, 1::2] = -sin * g
    return (np.ascontiguousarray(ct.reshape(S, n_heads * HD), dtype=np.float16),
            np.ascontiguousarray(st.reshape(S, n_heads * HD), dtype=np.float16))


def _in_map_for_core(x, Wq, Wk, Wv, Wo, q_gain, core):
    b, hh = core // 2, core % 2
    lq0 = HL * hh                         # first local q head (global index)
    kvh = slice(JKV * hh, JKV * (hh + 1))

    # Wq rows in (block j: head j, head j+4) order, pair-interleaved feats
    qrows = np.concatenate([64 * (lq0 + h) + IVF for h in QBLK])
    # Wk rows pair-interleaved per kv head; Wv rows plain
    krows = np.concatenate([64 * u + IVF for u in range(KVL)])
    wkv = np.concatenate([Wk[kvh, :][krows, :], Wv[kvh, :]], axis=0)
    # Wo cols for y slot order
    orows = np.concatenate([64 * (lq0 + h) + np.arange(64) for h in YSLOT])

    gains = q_gain[[lq0 + h for h in QBLK]]
    cq, sq = _rope_tables(HL, gains)
    ck, sk = _rope_tables(KVL, np.ones(KVL, dtype=np.float32))
    return {
        "xT": np.ascontiguousarray(x[b].T),
        "wqT": np.ascontiguousarray(Wq[qrows, :].T),
        "wkvT": np.ascontiguousarray(wkv.T),
        "woT": np.ascontiguousarray(Wo[:, orows].T.astype(np.float16)),
        "cqi": cq, "sqi": sq, "cki": ck, "ski": sk,
    }


def kernel(x, Wq, Wk, Wv, Wo, q_gain):
    x = np.asarray(x, dtype=np.float32)
    Wq = np.asarray(Wq, dtype=np.float32)
    Wk = np.asarray(Wk, dtype=np.float32)
    Wv = np.asarray(Wv, dtype=np.float32)
    Wo = np.asarray(Wo, dtype=np.float32)
    q_gain = np.asarray(q_gain, dtype=np.float32)

    if "nc" not in _PROGRAM_CACHE:
        _PROGRAM_CACHE["nc"] = _build_program()
    nc = _PROGRAM_CACHE["nc"]

    in_maps = [_in_map_for_core(x, Wq, Wk, Wv, Wo, q_gain, core)
               for core in range(N_CORES)]

    res = run_bass_kernel_spmd(nc, in_maps, core_ids=list(range(N_CORES)))
    _PROGRAM_CACHE["last_results"] = res

    out = np.empty((B, S, D), dtype=np.float32)
    for b in range(B):
        out[b] = res.results[2 * b]["outp"] + res.results[2 * b + 1]["outp"]
    return out


if __name__ == "__main__":
    rng = np.random.default_rng(0)
    inputs = {
        "x": rng.standard_normal((B, S, D), dtype=np.float32),
        "Wq": rng.standard_normal((D, D), dtype=np.float32) * 0.02,
        "Wk": rng.standard_normal((KVH * HD, D), dtype=np.float32) * 0.02,
        "Wv": rng.standard_normal((KVH * HD, D), dtype=np.float32) * 0.02,
        "Wo": rng.standard_normal((D, D), dtype=np.float32) * 0.02,
        "q_gain": np.full((H,), 1.5, dtype=np.float32),
    }
    out = kernel(**inputs)
    print(out.shape, out.dtype, np.abs(out).max())


# revision 51
# speedup vs baseline: 1.2541x; 1.0008x over previous
"""Trainium2 Bass kernel for a dense transformer attention block.

Reference computation (fp32):
  q = rms_norm(x @ Wq.T)  per head (16 heads x 64)  -> rope -> * q_gain
  k = rms_norm(x @ Wk.T)  per kv-head (4 x 64)      -> rope
  v = x @ Wv.T
  causal GQA attention (16 q heads over 4 kv heads), softmax(q k / 8)
  out = (attn @ v) @ Wo.T

Sharding over 8 cores: core c = 2*b + hh handles batch b (of 4) and
q-head half hh (8 q heads = 2 kv heads).  Each core produces a partial
out [2048, 1024] (its heads' contribution through Wo); the host adds
the two partials per batch.  No collectives.

Datapath design (tuned against the TimelineSim cost model):
  - projections run in fp32r (full speed at free-size>=256); everything
    downstream (q/k/v, rope tables, probabilities, y, Wo) is fp16, which
    doubles DVE throughput (2x_1p) and keeps all matmuls at 1 cycle/row
  - q/k features are pair-interleaved (slot 2m <- feat m, 2m+1 <- feat
    32+m) so rope is 3 wide DVE ops using a stride -1 pair-swap view;
    the rotation sign lives in the host-built sin table; q_gain is
    folded into the q cos/sin tables
  - RMS factors: fp16 square + one grouped reduce per tile, then a
    Newton rsqrt batched over 2 token tiles (~10 small DVE ops)
  - scores are built transposed ([k, q]) so softmax needs no transposes:
    exp(s/8 - 4) is fused into the ACT evacuation of the QK psum (2
    k-tiles per instruction), the denominator comes from a ones column
    in v, and the normalization is applied per-partition post-PV
  - causal frontier masking and the rms scale/rope of k run on the
    otherwise-idle GPSIMD engine

Scheduling: all three stages are emitted through a quantum scheduler —
QK score tiles (the pacers of the scalar engine's ~157us exp stream)
alternate with metered filler (PV of earlier heads, stage-1 batches for
later q-groups, deferred output projections), so the exp stream starts
early and the 8 PSUM banks stay within budget via shared tag rings.
"""

import hashlib
import os

import numpy as np

# The libneuronxla NEFF cache can key-collide across different kernel
# versions with identical I/O shapes (observed: a stale NEFF served for an
# edited kernel).  Key the cache by this file's content so a changed kernel
# never hits a stale entry while identical re-runs stay warm.
try:
    _SRC_HASH = hashlib.sha256(open(__file__, "rb").read()).hexdigest()[:16]
except OSError:
    _SRC_HASH = "nosrc"
os.environ["NEURON_COMPILE_CACHE_URL"] = os.path.join(
    os.environ.get("TMPDIR", "/tmp"), f"neuron-cache-{_SRC_HASH}")

import concourse.bass as bass
import concourse.mybir as mybir
import concourse.tile as tile
from concourse import bacc
from concourse.bass_utils import run_bass_kernel_spmd
from concourse.masks import make_identity, make_upper_triangular

F32 = mybir.dt.float32
F32R = mybir.dt.float32r
F16 = mybir.dt.float16
AFT = mybir.ActivationFunctionType
ALU = mybir.AluOpType

B, S, D = 4, 2048, 1024
H, HD, KVH = 16, 64, 4
HL = 8            # q heads per core
KVL = 2           # kv heads per core
JQ = HL * HD      # 512 q-proj cols per core
JKV = KVL * HD    # 128 k (or v) proj cols per core
TT = S // 128     # 16 token tiles
DT = D // 128     # 8 contraction tiles
G = 4             # q groups of 512
ROPE_BASE = 10000.0
EPS = 1e-6
N_CORES = 8


def _build_program():
    nc = bacc.Bacc("TRN2", target_bir_lowering=False, debug=False,
                   num_devices=N_CORES)

    xT = nc.dram_tensor("xT", [D, S], F32R, kind="ExternalInput").ap()
    wqT = nc.dram_tensor("wqT", [D, JQ], F32R, kind="ExternalInput").ap()
    wkvT = nc.dram_tensor("wkvT", [D, 2 * JKV], F32R, kind="ExternalInput").ap()
    woT = nc.dram_tensor("woT", [JQ, D], F16, kind="ExternalInput").ap()
    cqi = nc.dram_tensor("cqi", [S, JQ], F16, kind="ExternalInput").ap()
    sqi = nc.dram_tensor("sqi", [S, JQ], F16, kind="ExternalInput").ap()
    cki = nc.dram_tensor("cki", [S, JKV], F16, kind="ExternalInput").ap()
    ski = nc.dram_tensor("ski", [S, JKV], F16, kind="ExternalInput").ap()
    outp = nc.dram_tensor("outp", [S, D], F32, kind="ExternalOutput").ap()
    xTr = xT.rearrange("(dt p) s -> p dt s", p=128)

    with tile.TileContext(nc) as tc:
        with (
            tc.tile_pool(name="consts", bufs=1) as consts,
            tc.tile_pool(name="persist", bufs=1) as persist,
        ):
            ident = consts.tile([128, 128], F16)
            make_identity(nc, ident)
            mask01 = consts.tile([128, 128], F16)
            make_upper_triangular(nc, mask01, val=1.0, diag=True)
            bias_m4 = consts.tile([128, 1], F32)
            nc.gpsimd.memset(bias_m4[:], -4.0)

            wq_sb = persist.tile([128, DT, JQ], F32R)
            wkv_sb = persist.tile([128, DT, 2 * JKV], F32R)
            wo_sb = persist.tile([128, JQ // 128, D], F16)
            qT_sb = persist.tile([128, 4, S], F16)
            kT_sb = persist.tile([128, S], F16)
            v_sb = persist.tile([128, TT, 2 * (HD + 1)], F16)
            stats = persist.tile([128, TT, HL + KVL], F32)
            r_all = persist.tile([128, TT, HL + KVL], F32)

            nc.sync.dma_start(
                wq_sb[:],
                wqT.rearrange("(dt p) j -> p dt j", p=128))
            nc.sync.dma_start(
                wkv_sb[:],
                wkvT.rearrange("(dt p) j -> p dt j", p=128))
            nc.sync.dma_start(
                wo_sb[:],
                woT.rearrange("(ft p) j -> p ft j", p=128))

            # ones columns of v (softmax denominator comes out of the PV
            # matmul)
            nc.gpsimd.memset(v_sb[:, :, HD:HD + 1], 1.0)
            nc.gpsimd.memset(v_sb[:, :, 2 * HD + 1:2 * HD + 2], 1.0)

            # Everything below shares one pool scope: stage 1 (projections /
            # rms / rope / transposes), stage 2 (attention) and stage 3
            # (output projection) are interleaved per q-group so the scalar
            # engine's exp stream — the global bottleneck — starts early and
            # never starves.  PSUM is only 8 banks, so the three stages share
            # tag rings: psA (2 banks x2) carries qkv-proj, score and out-proj
            # accumulators; psB (1 bank x2) carries every 128x128 transpose.
            with (
                tc.tile_pool(name="s1", bufs=2) as s1,
                tc.tile_pool(name="s2", bufs=2) as s2,
                tc.tile_pool(name="psA", bufs=2, space="PSUM") as psA,
                tc.tile_pool(name="psQ", bufs=1, space="PSUM") as psQ,
                tc.tile_pool(name="psB", bufs=1, space="PSUM") as psB,
                tc.tile_pool(name="psY", bufs=1, space="PSUM") as psY,
            ):
                held = {}     # tt -> (x_t, q_sb, k_sb) between sub-stages

                def load_x(tt):
                    _LABELS.append((('load_x', tt), nc.next_id()))
                    x_t = s1.tile([128, DT, 128], F32R, tag="x_t", bufs=4)
                    nc.sync.dma_start(x_t[:], xTr[:, :, 128 * tt:128 * (tt + 1)])
                    tsl = slice(128 * tt, 128 * (tt + 1))
                    cq = s1.tile([128, JQ], F16, tag="cq", bufs=4)
                    nc.sync.dma_start(cq[:], cqi[tsl, :])
                    sq = s1.tile([128, JQ], F16, tag="sq", bufs=4)
                    nc.sync.dma_start(sq[:], sqi[tsl, :])
                    ck = s1.tile([128, JKV], F16, tag="ck", bufs=4)
                    nc.sync.dma_start(ck[:], cki[tsl, :])
                    sk = s1.tile([128, JKV], F16, tag="sk", bufs=4)
                    nc.sync.dma_start(sk[:], ski[tsl, :])
                    held[tt] = [x_t, cq, sq, ck, sk]

                def stage1_a(tt):
                    _LABELS.append((('s1a', tt), nc.next_id()))
                    """Projections + evacuations + rms statistics."""
                    x_t = held[tt][0]
                    ps = psQ.tile([128, 2, 512], F32, tag="psQ", bufs=1)
                    psq, pskv = ps[:, 0, :], ps[:, 1, 0:2 * JKV]
                    for dt in range(DT):
                        nc.tensor.matmul(psq, x_t[:, dt, :], wq_sb[:, dt, :],
                                         start=(dt == 0), stop=(dt == DT - 1))
                    for dt in range(DT):
                        nc.tensor.matmul(pskv, x_t[:, dt, :], wkv_sb[:, dt, :],
                                         start=(dt == 0), stop=(dt == DT - 1))

                    # q_sb/k_sb are held across the 4-tile Newton batch, so
                    # their rings need 4 live buffers
                    q_sb = s1.tile([128, JQ], F16, tag="q_sb", bufs=4)
                    nc.scalar.copy(q_sb[:], psq)
                    k_sb = s1.tile([128, JKV], F16, tag="k_sb", bufs=4)
                    nc.vector.tensor_copy(k_sb[:], pskv[:, 0:JKV])
                    nc.scalar.copy(
                        v_sb[:, tt, :].rearrange("p (u f) -> p u f", u=KVL)
                        [:, :, 0:HD],
                        pskv[:, JKV:2 * JKV].rearrange("p (u f) -> p u f",
                                                       u=KVL))

                    # rms statistics: bn_stats gives [cnt, mean, M2] for the
                    # even and odd halves of each head's 64 features.  Read
                    # the SBUF copies, not the psum, so the psQ ring is freed
                    # by the (fast) evacuations alone.
                    nc.vector.bn_stats(
                        stats[:, tt, 0:HL, :],
                        q_sb.rearrange("p (h f) -> p h f", h=HL))
                    nc.vector.bn_stats(
                        stats[:, tt, HL:HL + KVL, :],
                        k_sb.rearrange("p (h f) -> p h f", h=KVL))
                    held[tt][0] = None
                    held[tt].extend([q_sb, k_sb])

                def newton(k):
                    _LABELS.append((('newton', k), nc.next_id()))
                    """r = (sumsq/64 + eps)^-0.5 for tiles 2k..2k+1.

                    Newton on DVE (seed y0=(a+b*m)/m has <=13% rel err for
                    m in [0.1,2]; 3 iterations -> ~1e-6).
                    """
                    st = stats[:, 2 * k:2 * (k + 1), :].unsqueeze(3)
                    m = s1.tile([128, 2, HL + KVL, 1], F32, tag="nw_m")
                    # m = sumsq/64 + eps
                    nc.vector.tensor_scalar(m[:], st, 1.0 / HD, EPS,
                                            ALU.mult, ALU.add)
                    w = s1.tile([128, 2, HL + KVL, 1], F32, tag="nw_w")
                    nc.vector.reciprocal(w[:], m[:])
                    r = r_all[:, 2 * k:2 * (k + 1), :].unsqueeze(3)
                    nc.vector.tensor_scalar(r, m[:], 0.657, 0.294,
                                            ALU.mult, ALU.add)
                    nc.vector.tensor_mul(r, r, w[:])
                    nt = s1.tile([128, 2, HL + KVL, 1], F32, tag="nw_nt")
                    for _ in range(3):
                        nc.vector.tensor_mul(nt[:], r, r)
                        nc.vector.tensor_mul(nt[:], nt[:], m[:])
                        nc.vector.tensor_scalar(nt[:], nt[:], -0.5, 1.5,
                                                ALU.mult, ALU.add)
                        nc.vector.tensor_mul(r, r, nt[:])

                def s1b_rope(tt):
                    _LABELS.append((('s1b', tt), nc.next_id()))
                    """Apply rms scale and rope (DVE/Pool only)."""
                    _, cq, sq, ck, sk, q_sb, k_sb = held[tt]
                    # scale q rows by their rms factors (Pool; in place)
                    for h in range(HL):
                        nc.gpsimd.tensor_scalar_mul(
                            q_sb[:, 64 * h:64 * (h + 1)],
                            q_sb[:, 64 * h:64 * (h + 1)],
                            r_all[:, tt, h:h + 1])
                    for u in range(KVL):
                        nc.gpsimd.tensor_scalar_mul(
                            k_sb[:, 64 * u:64 * (u + 1)],
                            k_sb[:, 64 * u:64 * (u + 1)],
                            r_all[:, tt, HL + u:HL + u + 1])

                    def rope(dst, src, cos_t, sin_t, tmp, eng):
                        swap = src.rearrange("p (a two) -> p a two", two=2)
                        eng.tensor_mul(
                            tmp.rearrange("p (a two) -> p a two", two=2),
                            swap[:, :, ::-1],
                            sin_t.rearrange("p (a two) -> p a two", two=2))
                        eng.tensor_mul(dst, src, cos_t)
                        eng.tensor_add(dst, dst, tmp)

                    qr = s1.tile([128, JQ], F16, tag="qr", bufs=4)
                    tmpq = s1.tile([128, JQ], F16, tag="tmpq", bufs=4)
                    rope(qr[:], q_sb[:], cq[:], sq[:], tmpq[:], nc.vector)
                    kr = s1.tile([128, JKV], F16, tag="kr", bufs=4)
                    tmpk = s1.tile([128, JKV], F16, tag="tmpk", bufs=4)
                    rope(kr[:], k_sb[:], ck[:], sk[:], tmpk[:], nc.gpsimd)
                    held[tt].extend([qr, kr])

                def s1b_transpose(tt):
                    """Transpose rope output into feature-major qT/kT.

                    q feature block j holds [head j of kv0 | head j+4 of
                    kv1], so transposed partitions align with kT's kv
                    blocks at offsets {0,64}."""
                    qr, kr = held.pop(tt)[-2:]
                    tsl = slice(128 * tt, 128 * (tt + 1))
                    ptr = psB.tile([128, 4, 128], F16, tag="psB")
                    for j in range(4):
                        nc.tensor.transpose(ptr[:, j, :],
                                            qr[:, 128 * j:128 * (j + 1)],
                                            ident[:])
                    nc.vector.tensor_copy(qT_sb[:, :, tsl], ptr[:])
                    ptk = psB.tile([128, 4, 128], F16, tag="psB", name="ptk")
                    nc.tensor.transpose(ptk[:, 0, :], kr[:], ident[:])
                    nc.vector.tensor_copy(kT_sb[:, tsl], ptk[:, 0, :])

                def s1_batch_gen(b):
                    """Full stage 1 for tiles 4b..4b+3, as scheduler quanta.

                    Yields the approximate PE-ns of each emitted quantum so
                    the driver can meter PE filler between QK score tiles.
                    """
                    tiles = range(4 * b, 4 * b + 4)
                    for tt in tiles:
                        load_x(tt)
                        yield 0

                    def s1a(tt):
                        _LABELS.append((('s1a', tt), nc.next_id()))
                        x_t = held[tt][0]
                        # separate single-bank rings: proj-q of tile t+1 only
                        # waits on the q evacuation of tile t
                        psq = psQ.tile([128, 512], F32, tag="psq", bufs=1)
                        pskv = psQ.tile([128, 2 * JKV], F32, tag="pskv",
                                        bufs=1)
                        for half in range(2):
                            for dt in range(4 * half, 4 * half + 4):
                                nc.tensor.matmul(
                                    psq[:], x_t[:, dt, :], wq_sb[:, dt, :],
                                    start=(dt == 0), stop=(dt == DT - 1),
                                    skip_group_check=True)
                            yield 854
                        for half in range(2):
                            for dt in range(4 * half, 4 * half + 4):
                                nc.tensor.matmul(
                                    pskv[:], x_t[:, dt, :], wkv_sb[:, dt, :],
                                    start=(dt == 0), stop=(dt == DT - 1),
                                    skip_group_check=True)
                            yield 427
                        # evacuations (all DVE; ACT stays a pure exp stream)
                        # and rms statistics off the SBUF copies
                        q_sb = s1.tile([128, JQ], F16, tag="q_sb", bufs=4)
                        nc.vector.tensor_copy(q_sb[:], psq[:])
                        k_sb = s1.tile([128, JKV], F16, tag="k_sb", bufs=4)
                        nc.scalar.copy(k_sb[:], pskv[:, 0:JKV])
                        nc.scalar.copy(
                            v_sb[:, tt, :].rearrange("p (u f) -> p u f",
                                                     u=KVL)[:, :, 0:HD],
                            pskv[:, JKV:2 * JKV].rearrange(
                                "p (u f) -> p u f", u=KVL))
                        sq = s1.tile([128, JQ + JKV], F32, tag="sqsc", name="sqsc")
                        nc.scalar.activation(sq[:, 0:JQ], psq[:], AFT.Square)
                        nc.scalar.activation(sq[:, JQ:JQ + JKV],
                                             pskv[:, 0:JKV], AFT.Square)
                        nc.vector.reduce_sum(
                            out=stats[:, tt, :].unsqueeze(2),
                            in_=sq.rearrange("p (h f) -> p h f", h=HL + KVL),
                            axis=mybir.AxisListType.X)
                        held[tt][0] = None
                        held[tt].extend([q_sb, k_sb])

                    # 2-tile sub-batches so ropes unblock early: the Newton
                    # rsqrt chain only gates two tiles at a time
                    for half in range(2):
                        sub = list(tiles)[2 * half:2 * half + 2]
                        for tt in sub:
                            for q in s1a(tt):
                                yield q
                            yield 0
                        newton(2 * b + half)
                        yield 0
                        for tt in sub:
                            s1b_rope(tt)
                            s1b_transpose(tt)
                            yield 265

                def qk_gen(g, s, expT):
                    """Scores + exp for head slot s, one psum tile at a time."""
                    _LABELS.append((('qk', g, s), nc.next_id()))
                    u, j = s % 2, s // 2
                    qrhs = qT_sb[64 * u:64 * (u + 1), j,
                                 512 * g:512 * (g + 1)]
                    # full rectangle k-tiles, 2 per psum tile; exp fused into
                    # the ACT evacuation (2 k-tiles per instruction)
                    for c in range(2 * g):
                        pss = psA.tile([128, 2, 512], F32, tag="psA",
                                       name="pss")
                        for lane in range(2):
                            kt = 2 * c + lane
                            nc.tensor.matmul(
                                pss[:, lane, :],
                                kT_sb[64 * u:64 * (u + 1),
                                      128 * kt:128 * (kt + 1)],
                                qrhs)
                        nc.scalar.activation(expT[:, 2 * c:2 * c + 2, :],
                                             pss[:], AFT.Exp,
                                             scale=0.125, bias=bias_m4[:])
                        yield 427
                    # diagonal k-tiles (causal frontier); the sub-diagonal
                    # 128-col block of each is masked on GPSIMD
                    for dc in range(2):
                        pss = psA.tile([128, 2, 512], F32, tag="psA",
                                       name="pss")
                        for lane in range(2):
                            kt = 4 * g + 2 * dc + lane
                            n0 = 128 * (2 * dc + lane)
                            nc.tensor.matmul(
                                pss[:, lane, n0:512],
                                kT_sb[64 * u:64 * (u + 1),
                                      128 * kt:128 * (kt + 1)],
                                qrhs[:, n0:512])
                            nc.scalar.activation(expT[:, kt, n0:512],
                                                 pss[:, lane, n0:512],
                                                 AFT.Exp,
                                                 scale=0.125, bias=bias_m4[:])
                            nc.gpsimd.tensor_mul(expT[:, kt, n0:n0 + 128],
                                                 expT[:, kt, n0:n0 + 128],
                                                 mask01[:])
                        yield 374

                def pv_gen(g, s, expT, y_sb):
                    """PV + softmax normalization for head slot s."""
                    _LABELS.append((('pv', g, s), nc.next_id()))
                    u = s % 2
                    psy = psY.tile([128, 4, HD + 1], F32, tag="psy")
                    for i in range(4):
                        nkt = 4 * g + i + 1
                        for kt in range(nkt):
                            nc.tensor.matmul(
                                psy[:, i, :],
                                expT[:, kt, 128 * i:128 * (i + 1)],
                                v_sb[:, kt, (HD + 1) * u:(HD + 1) * (u + 1)],
                                start=(kt == 0), stop=(kt == nkt - 1))
                        yield 27 * nkt
                    # one fast evacuation frees the psY bank; normalize
                    # afterwards from SBUF (4x-mode TSPs)
                    y_un = s2.tile([128, 4, HD + 1], F16, tag="y_un")
                    nc.vector.tensor_copy(y_un[:], psy[:])
                    rl = s2.tile([128, 4, 1], F32, tag="rl")
                    nc.vector.reciprocal(rl[:], y_un[:, :, HD:HD + 1])
                    for i in range(4):
                        nc.vector.tensor_scalar_mul(
                            y_sb[:, i, 64 * s:64 * (s + 1)],
                            y_un[:, i, 0:HD], rl[:, i, :])
                    yield 0

                def s3_gen(g, y_sb):
                    """Output projection for q-group g (4 row blocks)."""
                    for i in range(4):
                        _LABELS.append((('s3', g, i), nc.next_id()))
                        ptt = psB.tile([128, 4, 128], F16, tag="psB",
                                       name="ptt")
                        for ft in range(4):
                            nc.tensor.transpose(
                                ptt[:, ft, :],
                                y_sb[:, i, 128 * ft:128 * (ft + 1)],
                                ident[:])
                        yT = s2.tile([128, 4, 128], F16, tag="yT")
                        nc.vector.tensor_copy(yT[:], ptt[:])
                        yield 212
                        pso = psA.tile([128, 2, 512], F32, tag="psA",
                                       name="pso")
                        for nt in range(2):
                            for ft in range(4):
                                nc.tensor.matmul(
                                    pso[:, nt, :], yT[:, ft, :],
                                    wo_sb[:, ft, 512 * nt:512 * (nt + 1)],
                                    start=(ft == 0), stop=(ft == 3))
                            yield 854
                        out_sb = s2.tile([128, D], F32, tag="out_sb")
                        nc.vector.tensor_copy(out_sb[:], pso[:])
                        r0 = 512 * g + 128 * i
                        nc.sync.dma_start(outp[r0:r0 + 128, :], out_sb[:])
                        yield 0

                # ---- driver: emit QK score tiles (the ACT pacers) round-
                # robined with metered PE filler from the deferred queues.
                from collections import deque
                bulk = deque()     # stage-1 batches and stage-3 groups
                prio = deque()     # PV generators (free the expT ring)

                s1_gens = {}

                def drain(gen):
                    for _ in gen:
                        pass

                def pump(target):
                    got = 0
                    while got < target and (prio or bulk):
                        q = prio[0] if prio else bulk[0]
                        try:
                            got += next(q)
                        except StopIteration:
                            if prio and q is prio[0]:
                                prio.popleft()
                            else:
                                bulk.popleft()
                    return got

                # prologue: stage 1 for tiles 0-3 (q-group 0's span);
                # later batches are queued up front and pumped as filler
                drain(s1_batch_gen(0))
                for b in range(1, 4):
                    s1_gens[b] = s1_batch_gen(b)
                    bulk.append(s1_gens[b])

                ys = {}
                pv_gens = {}
                for hi, (g, s) in enumerate(
                        (g, s) for g in range(G) for s in range(HL)):
                    if s == 0:
                        # tiles 4g..4g+3 must be fully emitted before this
                        # group's QKs reference qT/kT (emission order is
                        # engine program order)
                        if g in s1_gens:
                            drain(s1_gens.pop(g))
                        # stage 3 is deferred into later, ACT-heavier groups
                        if g == 2:
                            bulk.append(s3_gen(0, ys[0]))
                        if g == 3:
                            bulk.append(s3_gen(1, ys[1]))
                            bulk.append(s3_gen(2, ys[2]))
                        ys[g] = s2.tile([128, 4, JQ], F16, tag="y_sb",
                                        bufs=4, name="y_sb")
                    # the expT ring is 2 deep: pv(hi-2) must be fully
                    # emitted before expT[hi] is allocated over its slot
                    if hi - 2 in pv_gens:
                        drain(pv_gens.pop(hi - 2))
                    expT_h = s2.tile([128, 4 * g + 4, 512], F16,
                                     tag="expT", name="expT")
                    for cost in qk_gen(g, s, expT_h):
                        pump(int(cost * 1.0))
                    pv_gens[hi] = pv_gen(g, s, expT_h, ys[g])
                    prio.append(pv_gens[hi])
                # tail: remaining PV, deferred work, then the last stage 3
                for k in sorted(pv_gens):
                    drain(pv_gens.pop(k))
                while prio or bulk:
                    pump(1 << 30)
                drain(s3_gen(3, ys[3]))

    nc.compile()
    return nc


_PROGRAM_CACHE = {}
_LABELS = []

# within-head feature interleave: slot 2m <- feat m, slot 2m+1 <- feat 32+m
IVF = np.empty(HD, dtype=np.int64)
IVF[0::2] = np.arange(32)
IVF[1::2] = np.arange(32, 64)

# q-head slot order: feature block j holds heads (j, j+4) = (j of kv0,
# j of kv1); y slot s holds head (s//2) + 4*(s%2)
QBLK = [0, 4, 1, 5, 2, 6, 3, 7]      # feature order for Wq cols / rope
YSLOT = [0, 4, 1, 5, 2, 6, 3, 7]     # y_sb slot s -> local head


def _rope_tables(n_heads, gains):
    """Pair-interleaved cos/sin tables [S, n_heads*64] with the rotation
    sign folded into sin: slot 2m gets (cos, sin), slot 2m+1 (cos, -sin)."""
    inv_freq = 1.0 / (ROPE_BASE ** (np.arange(0, HD, 2, dtype=np.float32) / HD))
    t = np.arange(S, dtype=np.float32)
    freqs = np.outer(t, inv_freq)                    # [S, 32]
    cos, sin = np.cos(freqs), np.sin(freqs)
    ct = np.empty((S, n_heads, HD), dtype=np.float32)
    st = np.empty((S, n_heads, HD), dtype=np.float32)
    for h in range(n_heads):
        g = gains[h]
        ct[:, h, 0::2] = cos * g
        ct[:, h, 1::2] = cos * g
        st[:, h, 0::2] = sin * g
        st[:, h, 1::2] = -sin * g
    return (np.ascontiguousarray(ct.reshape(S, n_heads * HD), dtype=np.float16),
            np.ascontiguousarray(st.reshape(S, n_heads * HD), dtype=np.float16))


def _in_map_for_core(x, Wq, Wk, Wv, Wo, q_gain, core):
    b, hh = core // 2, core % 2
    lq0 = HL * hh                         # first local q head (global index)
    kvh = slice(JKV * hh, JKV * (hh + 1))

    # Wq rows in (block j: head j, head j+4) order, pair-interleaved feats
    qrows = np.concatenate([64 * (lq0 + h) + IVF for h in QBLK])
    # Wk rows pair-interleaved per kv head; Wv rows plain
    krows = np.concatenate([64 * u + IVF for u in range(KVL)])
    wkv = np.concatenate([Wk[kvh, :][krows, :], Wv[kvh, :]], axis=0)
    # Wo cols for y slot order
    orows = np.concatenate([64 * (lq0 + h) + np.arange(64) for h in YSLOT])

    gains = q_gain[[lq0 + h for h in QBLK]]
    cq, sq = _rope_tables(HL, gains)
    ck, sk = _rope_tables(KVL, np.ones(KVL, dtype=np.float32))
    return {
        "xT": np.ascontiguousarray(x[b].T),
        "wqT": np.ascontiguousarray(Wq[qrows, :].T),
        "wkvT": np.ascontiguousarray(wkv.T),
        "woT": np.ascontiguousarray(Wo[:, orows].T.astype(np.float16)),
        "cqi": cq, "sqi": sq, "cki": ck, "ski": sk,
    }


def kernel(x, Wq, Wk, Wv, Wo, q_gain):
    x = np.asarray(x, dtype=np.float32)
    Wq = np.asarray(Wq, dtype=np.float32)
    Wk = np.asarray(Wk, dtype=np.float32)
    Wv = np.asarray(Wv, dtype=np.float32)
    Wo = np.asarray(Wo, dtype=np.float32)
    q_gain = np.asarray(q_gain, dtype=np.float32)

    if "nc" not in _PROGRAM_CACHE:
        _PROGRAM_CACHE["nc"] = _build_program()
    nc = _PROGRAM_CACHE["nc"]

    in_maps = [_in_map_for_core(x, Wq, Wk, Wv, Wo, q_gain, core)
               for core in range(N_CORES)]

    res = run_bass_kernel_spmd(nc, in_maps, core_ids=list(range(N_CORES)))
    _PROGRAM_CACHE["last_results"] = res

    out = np.empty((B, S, D), dtype=np.float32)
    for b in range(B):
        out[b] = res.results[2 * b]["outp"] + res.results[2 * b + 1]["outp"]
    return out


if __name__ == "__main__":
    rng = np.random.default_rng(0)
    inputs = {
        "x": rng.standard_normal((B, S, D), dtype=np.float32),
        "Wq": rng.standard_normal((D, D), dtype=np.float32) * 0.02,
        "Wk": rng.standard_normal((KVH * HD, D), dtype=np.float32) * 0.02,
        "Wv": rng.standard_normal((KVH * HD, D), dtype=np.float32) * 0.02,
        "Wo": rng.standard_normal((D, D), dtype=np.float32) * 0.02,
        "q_gain": np.full((H,), 1.5, dtype=np.float32),
    }
    out = kernel(**inputs)
    print(out.shape, out.dtype, np.abs(out).max())


# revision 54
# speedup vs baseline: 1.2628x; 1.0070x over previous
"""Trainium2 Bass kernel for a dense transformer attention block.

Reference computation (fp32):
  q = rms_norm(x @ Wq.T)  per head (16 heads x 64)  -> rope -> * q_gain
  k = rms_norm(x @ Wk.T)  per kv-head (4 x 64)      -> rope
  v = x @ Wv.T
  causal GQA attention (16 q heads over 4 kv heads), softmax(q k / 8)
  out = (attn @ v) @ Wo.T

Sharding over 8 cores: core c = 2*b + hh handles batch b (of 4) and
q-head half hh (8 q heads = 2 kv heads).  Each core produces a partial
out [2048, 1024] (its heads' contribution through Wo); the host adds
the two partials per batch.  No collectives.

Datapath design (v2, tuned against the TimelineSim cost model):
  - projections run in fp32r (full speed at free-size>=256); everything
    downstream (q/k/v, rope tables, probabilities, y, Wo) is fp16, which
    doubles DVE throughput (2x_1p) and keeps all matmuls at 1 cycle/row
  - q/k features are pair-interleaved (slot 2m <- feat m, 2m+1 <- feat
    32+m) so rope is 3 wide DVE ops using a stride -1 pair-swap view;
    the rotation sign lives in the host-built sin table; q_gain is
    folded into the q cos/sin tables
  - RMS statistics come from one bn_stats op per projection (even/odd
    mean/M2 6-tuples), assembled and pushed through a batched Newton
    rsqrt over 4 token tiles at a time (all fp32, ~13 small DVE ops)
  - scores are built transposed ([k, q]) so softmax needs no transposes:
    exp(s/8 - 4) is fused into the ACT evacuation of the QK psum (2
    k-tiles per instruction), the denominator comes from a ones column
    in v, and the normalization is applied per-partition post-PV
  - causal frontier masking runs on the otherwise-idle GPSIMD engine
  - stage-2 issues QK(s) one head ahead of PV(s-1) so the scalar engine
    (the global bottleneck: ~157us of exp) never starves
"""

import hashlib
import os

import numpy as np

# The libneuronxla NEFF cache can key-collide across different kernel
# versions with identical I/O shapes (observed: a stale NEFF served for an
# edited kernel).  Key the cache by this file's content so a changed kernel
# never hits a stale entry while identical re-runs stay warm.
try:
    _SRC_HASH = hashlib.sha256(open(__file__, "rb").read()).hexdigest()[:16]
except OSError:
    _SRC_HASH = "nosrc"
os.environ["NEURON_COMPILE_CACHE_URL"] = os.path.join(
    os.environ.get("TMPDIR", "/tmp"), f"neuron-cache-{_SRC_HASH}")

import concourse.bass as bass
import concourse.mybir as mybir
import concourse.tile as tile
from concourse import bacc
from concourse.bass_utils import run_bass_kernel_spmd
from concourse.masks import make_identity, make_upper_triangular

F32 = mybir.dt.float32
F32R = mybir.dt.float32r
F16 = mybir.dt.float16
AFT = mybir.ActivationFunctionType
ALU = mybir.AluOpType

B, S, D = 4, 2048, 1024
H, HD, KVH = 16, 64, 4
HL = 8            # q heads per core
KVL = 2           # kv heads per core
JQ = HL * HD      # 512 q-proj cols per core
JKV = KVL * HD    # 128 k (or v) proj cols per core
TT = S // 128     # 16 token tiles
DT = D // 128     # 8 contraction tiles
G = 4             # q groups of 512
ROPE_BASE = 10000.0
EPS = 1e-6
N_CORES = 8


def _build_program():
    nc = bacc.Bacc("TRN2", target_bir_lowering=False, debug=False,
                   num_devices=N_CORES)

    xT = nc.dram_tensor("xT", [D, S], F32R, kind="ExternalInput").ap()
    wqT = nc.dram_tensor("wqT", [D, JQ], F32R, kind="ExternalInput").ap()
    wkvT = nc.dram_tensor("wkvT", [D, 2 * JKV], F32R, kind="ExternalInput").ap()
    woT = nc.dram_tensor("woT", [JQ, D], F16, kind="ExternalInput").ap()
    cqi = nc.dram_tensor("cqi", [S, JQ], F16, kind="ExternalInput").ap()
    sqi = nc.dram_tensor("sqi", [S, JQ], F16, kind="ExternalInput").ap()
    cki = nc.dram_tensor("cki", [S, JKV], F16, kind="ExternalInput").ap()
    ski = nc.dram_tensor("ski", [S, JKV], F16, kind="ExternalInput").ap()
    outp = nc.dram_tensor("outp", [S, D], F32, kind="ExternalOutput").ap()
    xTr = xT.rearrange("(dt p) s -> p dt s", p=128)

    with tile.TileContext(nc) as tc:
        with (
            tc.tile_pool(name="consts", bufs=1) as consts,
            tc.tile_pool(name="persist", bufs=1) as persist,
        ):
            ident = consts.tile([128, 128], F16)
            make_identity(nc, ident)
            mask01 = consts.tile([128, 128], F16)
            make_upper_triangular(nc, mask01, val=1.0, diag=True)
            bias_m4 = consts.tile([128, 1], F32)
            nc.gpsimd.memset(bias_m4[:], -4.0)

            wq_sb = persist.tile([128, DT, JQ], F32R)
            wkv_sb = persist.tile([128, DT, 2 * JKV], F32R)
            wo_sb = persist.tile([128, JQ // 128, D], F16)
            qT_sb = persist.tile([128, 4, S], F16)
            kT_sb = persist.tile([128, S], F16)
            v_sb = persist.tile([128, TT, 2 * (HD + 1)], F16)
            stats = persist.tile([128, TT, HL + KVL], F32)
            r_all = persist.tile([128, TT, HL + KVL], F32)

            nc.sync.dma_start(
                wq_sb[:],
                wqT.rearrange("(dt p) j -> p dt j", p=128))
            nc.sync.dma_start(
                wkv_sb[:],
                wkvT.rearrange("(dt p) j -> p dt j", p=128))
            nc.sync.dma_start(
                wo_sb[:],
                woT.rearrange("(ft p) j -> p ft j", p=128))

            # ones columns of v (softmax denominator comes out of the PV
            # matmul)
            nc.gpsimd.memset(v_sb[:, :, HD:HD + 1], 1.0)
            nc.gpsimd.memset(v_sb[:, :, 2 * HD + 1:2 * HD + 2], 1.0)

            # Everything below shares one pool scope: stage 1 (projections /
            # rms / rope / transposes), stage 2 (attention) and stage 3
            # (output projection) are interleaved per q-group so the scalar
            # engine's exp stream — the global bottleneck — starts early and
            # never starves.  PSUM is only 8 banks, so the three stages share
            # tag rings: psA (2 banks x2) carries qkv-proj, score and out-proj
            # accumulators; psB (1 bank x2) carries every 128x128 transpose.
            with (
                tc.tile_pool(name="s1", bufs=2) as s1,
                tc.tile_pool(name="s2", bufs=2) as s2,
                tc.tile_pool(name="psA", bufs=2, space="PSUM") as psA,
                tc.tile_pool(name="psQ", bufs=1, space="PSUM") as psQ,
                tc.tile_pool(name="psB", bufs=1, space="PSUM") as psB,
                tc.tile_pool(name="psY", bufs=1, space="PSUM") as psY,
            ):
                held = {}     # tt -> (x_t, q_sb, k_sb) between sub-stages

                def load_x(tt):
                    _LABELS.append((('load_x', tt), nc.next_id()))
                    x_t = s1.tile([128, DT, 128], F32R, tag="x_t", bufs=4)
                    nc.sync.dma_start(x_t[:], xTr[:, :, 128 * tt:128 * (tt + 1)])
                    tsl = slice(128 * tt, 128 * (tt + 1))
                    cq = s1.tile([128, JQ], F16, tag="cq", bufs=4)
                    nc.sync.dma_start(cq[:], cqi[tsl, :])
                    sq = s1.tile([128, JQ], F16, tag="sq", bufs=4)
                    nc.sync.dma_start(sq[:], sqi[tsl, :])
                    ck = s1.tile([128, JKV], F16, tag="ck", bufs=4)
                    nc.sync.dma_start(ck[:], cki[tsl, :])
                    sk = s1.tile([128, JKV], F16, tag="sk", bufs=4)
                    nc.sync.dma_start(sk[:], ski[tsl, :])
                    held[tt] = [x_t, cq, sq, ck, sk]

                def stage1_a(tt):
                    _LABELS.append((('s1a', tt), nc.next_id()))
                    """Projections + evacuations + rms statistics."""
                    x_t = held[tt][0]
                    ps = psQ.tile([128, 2, 512], F32, tag="psQ", bufs=1)
                    psq, pskv = ps[:, 0, :], ps[:, 1, 0:2 * JKV]
                    for dt in range(DT):
                        nc.tensor.matmul(psq, x_t[:, dt, :], wq_sb[:, dt, :],
                                         start=(dt == 0), stop=(dt == DT - 1))
                    for dt in range(DT):
                        nc.tensor.matmul(pskv, x_t[:, dt, :], wkv_sb[:, dt, :],
                                         start=(dt == 0), stop=(dt == DT - 1))

                    # q_sb/k_sb are held across the 4-tile Newton batch, so
                    # their rings need 4 live buffers
                    q_sb = s1.tile([128, JQ], F16, tag="q_sb", bufs=4)
                    nc.scalar.copy(q_sb[:], psq)
                    k_sb = s1.tile([128, JKV], F16, tag="k_sb", bufs=4)
                    nc.vector.tensor_copy(k_sb[:], pskv[:, 0:JKV])
                    nc.scalar.copy(
                        v_sb[:, tt, :].rearrange("p (u f) -> p u f", u=KVL)
                        [:, :, 0:HD],
                        pskv[:, JKV:2 * JKV].rearrange("p (u f) -> p u f",
                                                       u=KVL))

                    # rms statistics: bn_stats gives [cnt, mean, M2] for the
                    # even and odd halves of each head's 64 features.  Read
                    # the SBUF copies, not the psum, so the psQ ring is freed
                    # by the (fast) evacuations alone.
                    nc.vector.bn_stats(
                        stats[:, tt, 0:HL, :],
                        q_sb.rearrange("p (h f) -> p h f", h=HL))
                    nc.vector.bn_stats(
                        stats[:, tt, HL:HL + KVL, :],
                        k_sb.rearrange("p (h f) -> p h f", h=KVL))
                    held[tt][0] = None
                    held[tt].extend([q_sb, k_sb])

                def newton(k):
                    _LABELS.append((('newton', k), nc.next_id()))
                    """r = (sumsq/64 + eps)^-0.5 for tiles 2k..2k+1.

                    Newton on DVE (seed y0=(a+b*m)/m has <=13% rel err for
                    m in [0.1,2]; 3 iterations -> ~1e-6).
                    """
                    st = stats[:, 2 * k:2 * (k + 1), :].unsqueeze(3)
                    m = s1.tile([128, 2, HL + KVL, 1], F32, tag="nw_m")
                    # m = sumsq/64 + eps
                    nc.vector.tensor_scalar(m[:], st, 1.0 / HD, EPS,
                                            ALU.mult, ALU.add)
                    w = s1.tile([128, 2, HL + KVL, 1], F32, tag="nw_w")
                    nc.vector.reciprocal(w[:], m[:])
                    r = r_all[:, 2 * k:2 * (k + 1), :].unsqueeze(3)
                    nc.vector.tensor_scalar(r, m[:], 0.657, 0.294,
                                            ALU.mult, ALU.add)
                    nc.vector.tensor_mul(r, r, w[:])
                    nt = s1.tile([128, 2, HL + KVL, 1], F32, tag="nw_nt")
                    for _ in range(3):
                        nc.vector.tensor_mul(nt[:], r, r)
                        nc.vector.tensor_mul(nt[:], nt[:], m[:])
                        nc.vector.tensor_scalar(nt[:], nt[:], -0.5, 1.5,
                                                ALU.mult, ALU.add)
                        nc.vector.tensor_mul(r, r, nt[:])

                def s1b_rope(tt):
                    _LABELS.append((('s1b', tt), nc.next_id()))
                    """Apply rms scale and rope (DVE/Pool only)."""
                    _, cq, sq, ck, sk, q_sb, k_sb = held[tt]
                    # scale q rows by their rms factors (Pool; in place)
                    for h in range(HL):
                        nc.gpsimd.tensor_scalar_mul(
                            q_sb[:, 64 * h:64 * (h + 1)],
                            q_sb[:, 64 * h:64 * (h + 1)],
                            r_all[:, tt, h:h + 1])
                    for u in range(KVL):
                        nc.gpsimd.tensor_scalar_mul(
                            k_sb[:, 64 * u:64 * (u + 1)],
                            k_sb[:, 64 * u:64 * (u + 1)],
                            r_all[:, tt, HL + u:HL + u + 1])

                    def rope(dst, src, cos_t, sin_t, tmp, eng):
                        swap = src.rearrange("p (a two) -> p a two", two=2)
                        eng.tensor_mul(
                            tmp.rearrange("p (a two) -> p a two", two=2),
                            swap[:, :, ::-1],
                            sin_t.rearrange("p (a two) -> p a two", two=2))
                        eng.tensor_mul(dst, src, cos_t)
                        eng.tensor_add(dst, dst, tmp)

                    qr = s1.tile([128, JQ], F16, tag="qr", bufs=4)
                    tmpq = s1.tile([128, JQ], F16, tag="tmpq", bufs=4)
                    rope(qr[:], q_sb[:], cq[:], sq[:], tmpq[:], nc.vector)
                    kr = s1.tile([128, JKV], F16, tag="kr", bufs=4)
                    tmpk = s1.tile([128, JKV], F16, tag="tmpk", bufs=4)
                    rope(kr[:], k_sb[:], ck[:], sk[:], tmpk[:], nc.gpsimd)
                    held[tt].extend([qr, kr])

                def s1b_transpose(tt):
                    """Transpose rope output into feature-major qT/kT.

                    q feature block j holds [head j of kv0 | head j+4 of
                    kv1], so transposed partitions align with kT's kv
                    blocks at offsets {0,64}."""
                    qr, kr = held.pop(tt)[-2:]
                    tsl = slice(128 * tt, 128 * (tt + 1))
                    ptr = psB.tile([128, 4, 128], F16, tag="psB")
                    for j in range(4):
                        nc.tensor.transpose(ptr[:, j, :],
                                            qr[:, 128 * j:128 * (j + 1)],
                                            ident[:])
                    nc.vector.tensor_copy(qT_sb[:, :, tsl], ptr[:])
                    ptk = psB.tile([128, 4, 128], F16, tag="psB", name="ptk")
                    nc.tensor.transpose(ptk[:, 0, :], kr[:], ident[:])
                    nc.vector.tensor_copy(kT_sb[:, tsl], ptk[:, 0, :])

                def s1_batch_gen(b):
                    """Full stage 1 for tiles 4b..4b+3, as scheduler quanta.

                    Yields the approximate PE-ns of each emitted quantum so
                    the driver can meter PE filler between QK score tiles.
                    """
                    tiles = range(4 * b, 4 * b + 4)
                    for tt in tiles:
                        load_x(tt)
                        yield 0

                    def s1a(tt):
                        _LABELS.append((('s1a', tt), nc.next_id()))
                        x_t = held[tt][0]
                        # separate single-bank rings: proj-q of tile t+1 only
                        # waits on the q evacuation of tile t
                        psq = psQ.tile([128, 512], F32, tag="psq", bufs=1)
                        pskv = psQ.tile([128, 2 * JKV], F32, tag="pskv",
                                        bufs=1)
                        for half in range(2):
                            for dt in range(4 * half, 4 * half + 4):
                                nc.tensor.matmul(
                                    psq[:], x_t[:, dt, :], wq_sb[:, dt, :],
                                    start=(dt == 0), stop=(dt == DT - 1),
                                    skip_group_check=True)
                            yield 854
                        for half in range(2):
                            for dt in range(4 * half, 4 * half + 4):
                                nc.tensor.matmul(
                                    pskv[:], x_t[:, dt, :], wkv_sb[:, dt, :],
                                    start=(dt == 0), stop=(dt == DT - 1),
                                    skip_group_check=True)
                            yield 427
                        # evacuations (all DVE; ACT stays a pure exp stream)
                        # and rms statistics off the SBUF copies
                        q_sb = s1.tile([128, JQ], F16, tag="q_sb", bufs=4)
                        nc.vector.tensor_copy(q_sb[:], psq[:])
                        k_sb = s1.tile([128, JKV], F16, tag="k_sb", bufs=4)
                        nc.scalar.copy(k_sb[:], pskv[:, 0:JKV])
                        nc.scalar.copy(
                            v_sb[:, tt, :].rearrange("p (u f) -> p u f",
                                                     u=KVL)[:, :, 0:HD],
                            pskv[:, JKV:2 * JKV].rearrange(
                                "p (u f) -> p u f", u=KVL))
                        sq = s1.tile([128, JQ + JKV], F32, tag="sqsc", name="sqsc")
                        nc.scalar.activation(sq[:, 0:JQ], psq[:], AFT.Square)
                        nc.scalar.activation(sq[:, JQ:JQ + JKV],
                                             pskv[:, 0:JKV], AFT.Square)
                        nc.vector.reduce_sum(
                            out=stats[:, tt, :].unsqueeze(2),
                            in_=sq.rearrange("p (h f) -> p h f", h=HL + KVL),
                            axis=mybir.AxisListType.X)
                        held[tt][0] = None
                        held[tt].extend([q_sb, k_sb])

                    # 2-tile sub-batches so ropes unblock early: the Newton
                    # rsqrt chain only gates two tiles at a time
                    for half in range(2):
                        sub = list(tiles)[2 * half:2 * half + 2]
                        for tt in sub:
                            for q in s1a(tt):
                                yield q
                            yield 0
                        newton(2 * b + half)
                        yield 0
                        for tt in sub:
                            s1b_rope(tt)
                            s1b_transpose(tt)
                            yield 265

                def qk_gen(g, s, expT):
                    """Scores + exp for head slot s, one psum tile at a time."""
                    _LABELS.append((('qk', g, s), nc.next_id()))
                    u, j = s % 2, s // 2
                    qrhs = qT_sb[64 * u:64 * (u + 1), j,
                                 512 * g:512 * (g + 1)]
                    # full rectangle k-tiles, 2 per psum tile; exp fused into
                    # the ACT evacuation (2 k-tiles per instruction)
                    for c in range(2 * g):
                        pss = psA.tile([128, 2, 512], F32, tag="psA",
                                       name="pss")
                        for lane in range(2):
                            kt = 2 * c + lane
                            nc.tensor.matmul(
                                pss[:, lane, :],
                                kT_sb[64 * u:64 * (u + 1),
                                      128 * kt:128 * (kt + 1)],
                                qrhs)
                        nc.scalar.activation(expT[:, 2 * c:2 * c + 2, :],
                                             pss[:], AFT.Exp,
                                             scale=0.125, bias=bias_m4[:])
                        yield 427
                    # diagonal k-tiles (causal frontier); the sub-diagonal
                    # 128-col block of each is masked on GPSIMD
                    for dc in range(2):
                        pss = psA.tile([128, 2, 512], F32, tag="psA",
                                       name="pss")
                        for lane in range(2):
                            kt = 4 * g + 2 * dc + lane
                            n0 = 128 * (2 * dc + lane)
                            nc.tensor.matmul(
                                pss[:, lane, n0:512],
                                kT_sb[64 * u:64 * (u + 1),
                                      128 * kt:128 * (kt + 1)],
                                qrhs[:, n0:512])
                            nc.scalar.activation(expT[:, kt, n0:512],
                                                 pss[:, lane, n0:512],
                                                 AFT.Exp,
                                                 scale=0.125, bias=bias_m4[:])
                            nc.gpsimd.tensor_mul(expT[:, kt, n0:n0 + 128],
                                                 expT[:, kt, n0:n0 + 128],
                                                 mask01[:])
                        yield 374

                def pv_gen(g, s, expT, y_sb):
                    """PV + softmax normalization for head slot s."""
                    _LABELS.append((('pv', g, s), nc.next_id()))
                    u = s % 2
                    psy = psY.tile([128, 4, HD + 1], F32, tag="psy")
                    for i in range(4):
                        nkt = 4 * g + i + 1
                        for kt in range(nkt):
                            nc.tensor.matmul(
                                psy[:, i, :],
                                expT[:, kt, 128 * i:128 * (i + 1)],
                                v_sb[:, kt, (HD + 1) * u:(HD + 1) * (u + 1)],
                                start=(kt == 0), stop=(kt == nkt - 1))
                        yield 27 * nkt
                    # one fast evacuation frees the psY bank; normalize
                    # afterwards from SBUF (4x-mode TSPs)
                    y_un = s2.tile([128, 4, HD + 1], F16, tag="y_un")
                    nc.vector.tensor_copy(y_un[:], psy[:])
                    rl = s2.tile([128, 4, 1], F32, tag="rl")
                    nc.vector.reciprocal(rl[:], y_un[:, :, HD:HD + 1])
                    for i in range(4):
                        nc.vector.tensor_scalar_mul(
                            y_sb[:, i, 64 * s:64 * (s + 1)],
                            y_un[:, i, 0:HD], rl[:, i, :])
                    yield 0

                def s3_gen(g, y_sb):
                    """Output projection for q-group g (4 row blocks)."""
                    for i in range(4):
                        _LABELS.append((('s3', g, i), nc.next_id()))
                        ptt = psB.tile([128, 4, 128], F16, tag="psB",
                                       name="ptt")
                        for ft in range(4):
                            nc.tensor.transpose(
                                ptt[:, ft, :],
                                y_sb[:, i, 128 * ft:128 * (ft + 1)],
                                ident[:])
                        yT = s2.tile([128, 4, 128], F16, tag="yT")
                        nc.vector.tensor_copy(yT[:], ptt[:])
                        yield 212
                        pso = psA.tile([128, 2, 512], F32, tag="psA",
                                       name="pso")
                        for nt in range(2):
                            for ft in range(4):
                                nc.tensor.matmul(
                                    pso[:, nt, :], yT[:, ft, :],
                                    wo_sb[:, ft, 512 * nt:512 * (nt + 1)],
                                    start=(ft == 0), stop=(ft == 3))
                            yield 854
                        out_sb = s2.tile([128, D], F32, tag="out_sb")
                        nc.vector.tensor_copy(out_sb[:], pso[:])
                        r0 = 512 * g + 128 * i
                        nc.sync.dma_start(outp[r0:r0 + 128, :], out_sb[:])
                        yield 0

                # ---- driver: emit QK score tiles (the ACT pacers) round-
                # robined with metered PE filler from the deferred queues.
                from collections import deque
                bulk = deque()     # stage-1 batches and stage-3 groups
                prio = deque()     # PV generators (free the expT ring)

                s1_gens = {}

                def drain(gen):
                    for _ in gen:
                        pass

                def pump(target):
                    got = 0
                    while got < target and (prio or bulk):
                        q = prio[0] if prio else bulk[0]
                        try:
                            got += next(q)
                        except StopIteration:
                            if prio and q is prio[0]:
                                prio.popleft()
                            else:
                                bulk.popleft()
                    return got

                # prologue: stage 1 for tiles 0-3 (q-group 0's span);
                # later batches are queued up front and pumped as filler
                drain(s1_batch_gen(0))
                for b in range(1, 4):
                    s1_gens[b] = s1_batch_gen(b)
                    bulk.append(s1_gens[b])

                ys = {}
                pv_gens = {}
                for hi, (g, s) in enumerate(
                        (g, s) for g in range(G) for s in range(HL)):
                    if s == 0:
                        # tiles 4g..4g+3 must be fully emitted before this
                        # group's QKs reference qT/kT (emission order is
                        # engine program order)
                        if g in s1_gens:
                            drain(s1_gens.pop(g))
                        # stage 3 is deferred into later, ACT-heavier groups
                        if g == 2:
                            bulk.append(s3_gen(0, ys[0]))
                        if g == 3:
                            bulk.append(s3_gen(1, ys[1]))
                            bulk.append(s3_gen(2, ys[2]))
                        ys[g] = s2.tile([128, 4, JQ], F16, tag="y_sb",
                                        bufs=4, name="y_sb")
                    # the expT ring is 2 deep: pv(hi-2) must be fully
                    # emitted before expT[hi] is allocated over its slot
                    if hi - 2 in pv_gens:
                        drain(pv_gens.pop(hi - 2))
                    expT_h = s2.tile([128, 4 * g + 4, 512], F16,
                                     tag="expT", name="expT")
                    for cost in qk_gen(g, s, expT_h):
                        pump(int(cost * 1.0))
                    pv_gens[hi] = pv_gen(g, s, expT_h, ys[g])
                    prio.append(pv_gens[hi])
                # tail: remaining PV, deferred work, then the last stage 3
                for k in sorted(pv_gens):
                    drain(pv_gens.pop(k))
                while prio or bulk:
                    pump(1 << 30)
                drain(s3_gen(3, ys[3]))

    nc.compile()
    return nc


_PROGRAM_CACHE = {}
_LABELS = []

# within-head feature interleave: slot 2m <- feat m, slot 2m+1 <- feat 32+m
IVF = np.empty(HD, dtype=np.int64)
IVF[0::2] = np.arange(32)
IVF[1::2] = np.arange(32, 64)

# q-head slot order: feature block j holds heads (j, j+4) = (j of kv0,
# j of kv1); y slot s holds head (s//2) + 4*(s%2)
QBLK = [0, 4, 1, 5, 2, 6, 3, 7]      # feature order for Wq cols / rope
YSLOT = [0, 4, 1, 5, 2, 6, 3, 7]     # y_sb slot s -> local head


def _rope_tables(n_heads, gains):
    """Pair-interleaved cos/sin tables [S, n_heads*64] with the rotation
    sign folded into sin: slot 2m gets (cos, sin), slot 2m+1 (cos, -sin)."""
    inv_freq = 1.0 / (ROPE_BASE ** (np.arange(0, HD, 2, dtype=np.float32) / HD))
    t = np.arange(S, dtype=np.float32)
    freqs = np.outer(t, inv_freq)                    # [S, 32]
    cos, sin = np.cos(freqs), np.sin(freqs)
    ct = np.empty((S, n_heads, HD), dtype=np.float32)
    st = np.empty((S, n_heads, HD), dtype=np.float32)
    for h in range(n_heads):
        g = gains[h]
        ct[:, h, 0::2] = cos * g
        ct[:, h, 1::2] = cos * g
        st[:, h, 0::2] = sin * g
        st[:, h, 1::2] = -sin * g
    return (np.ascontiguousarray(ct.reshape(S, n_heads * HD), dtype=np.float16),
            np.ascontiguousarray(st.reshape(S, n_heads * HD), dtype=np.float16))


def _in_map_for_core(x, Wq, Wk, Wv, Wo, q_gain, core):
    b, hh = core // 2, core % 2
    lq0 = HL * hh                         # first local q head (global index)
    kvh = slice(JKV * hh, JKV * (hh + 1))

    # Wq rows in (block j: head j, head j+4) order, pair-interleaved feats
    qrows = np.concatenate([64 * (lq0 + h) + IVF for h in QBLK])
    # Wk rows pair-interleaved per kv head; Wv rows plain
    krows = np.concatenate([64 * u + IVF for u in range(KVL)])
    wkv = np.concatenate([Wk[kvh, :][krows, :], Wv[kvh, :]], axis=0)
    # Wo cols for y slot order
    orows = np.concatenate([64 * (lq0 + h) + np.arange(64) for h in YSLOT])

    gains = q_gain[[lq0 + h for h in QBLK]]
    cq, sq = _rope_tables(HL, gains)
    ck, sk = _rope_tables(KVL, np.ones(KVL, dtype=np.float32))
    return {
        "xT": np.ascontiguousarray(x[b].T),
        "wqT": np.ascontiguousarray(Wq[qrows, :].T),
        "wkvT": np.ascontiguousarray(wkv.T),
        "woT": np.ascontiguousarray(Wo[:, orows].T.astype(np.float16)),
        "cqi": cq, "sqi": sq, "cki": ck, "ski": sk,
    }


def kernel(x, Wq, Wk, Wv, Wo, q_gain):
    x = np.asarray(x, dtype=np.float32)
    Wq = np.asarray(Wq, dtype=np.float32)
    Wk = np.asarray(Wk, dtype=np.float32)
    Wv = np.asarray(Wv, dtype=np.float32)
    Wo = np.asarray(Wo, dtype=np.float32)
    q_gain = np.asarray(q_gain, dtype=np.float32)

    if "nc" not in _PROGRAM_CACHE:
        _PROGRAM_CACHE["nc"] = _build_program()
    nc = _PROGRAM_CACHE["nc"]

    in_maps = [_in_map_for_core(x, Wq, Wk, Wv, Wo, q_gain, core)
               for core in range(N_CORES)]

    res = run_bass_kernel_spmd(nc, in_maps, core_ids=list(range(N_CORES)))
    _PROGRAM_CACHE["last_results"] = res

    out = np.empty((B, S, D), dtype=np.float32)
    for b in range(B):
        out[b] = res.results[2 * b]["outp"] + res.results[2 * b + 1]["outp"]
    return out


if __name__ == "__main__":
    rng = np.random.default_rng(0)
    inputs = {
        "x": rng.standard_normal((B, S, D), dtype=np.float32),
        "Wq": rng.standard_normal((D, D), dtype=np.float32) * 0.02,
        "Wk": rng.standard_normal((KVH * HD, D), dtype=np.float32) * 0.02,
        "Wv": rng.standard_normal((KVH * HD, D), dtype=np.float32) * 0.02,
        "Wo": rng.standard_normal((D, D), dtype=np.float32) * 0.02,
        "q_gain": np.full((H,), 1.5, dtype=np.float32),
    }
    out = kernel(**inputs)
    print(out.shape, out.dtype, np.abs(out).max())
